# revision 43
# baseline (speedup 1.0000x reference)
"""
MLA attention (DeepSeek-style) on 8 TRN2 NeuronCores.

Sharding:
  phase 1 (LoRA-A projection + RMSNorm): sharded over sequence (256 rows/core),
    result transposed to feature-major and AllGathered (bf16 latents).
    The kv+rope latent columns are computed first and gathered in an early
    collective that overlaps the rest of phase 1; the q latents follow.
  phase 2 (q/kv up-proj, attention, o_proj): sharded over heads (4 heads/core),
    w_o input-dim sharded; partial outputs summed on the host (the all-reduce).

All heavy matmuls run in bf16 with fp32 PSUM accumulation.
Everything feature-major ("X^T" layout [feature, seq]) in phase 2 so no big
transposes are needed.

v2 changes vs baseline:
  - DMA queue spreading: weight streaming on sync+vector, collective staging
    on gpsimd (so gathers fire ~100us earlier), gathered latents pulled in on
    scalar+sync.
  - softmax row-sums via DVE accumulation of exp tiles + ONE M=1 matmul per
    (head, sq-block) instead of one per score tile (-74K PE columns).
  - the 1/rowsum broadcast matmul runs in bf16 (was fp32 = 4x column rate),
    reciprocal via the fast DVE approximation.
  - causal diagonal tiles trimmed to their valid column range (scores, exp,
    AV); the boundary 128-wide chunk is masked post-exp with a 0/1 bf16 mask.
  - rope-part q up-projection packed across head pairs (M=128 instead of 64).
  - partial outputs written bf16 (halves the output stream).
"""

import os
import sys
from contextlib import ExitStack

import numpy as np

for _p in ("/opt/trn_rl_repo", "/root/.axon_site/_ro/trn_rl_repo"):
    if os.path.isdir(_p) and _p not in sys.path:
        sys.path.insert(0, _p)

import ml_dtypes  # noqa: E402

import concourse.bacc as bacc  # noqa: E402
import concourse.bass as bass  # noqa: E402
import concourse.mybir as mybir  # noqa: E402
import concourse.tile as tile  # noqa: E402
from concourse.bass_utils import run_bass_kernel_spmd  # noqa: E402
from concourse.masks import make_identity  # noqa: E402

# ---------------------------------------------------------------- constants
NCORES = 8
S = 2048
SL = S // NCORES  # 256 local rows in phase 1
HID = 4096
Q_LORA = 1536
KV_LORA = 512
ROPE = 64
C = Q_LORA + KV_LORA + ROPE  # 2112
CKV_R = KV_LORA + ROPE  # 576 kv+rope latent cols
NOPE = 128
V_DIM = 128
H = 32
HL = H // NCORES  # 4 local heads
Q_HEAD = NOPE + ROPE  # 192
EPS = 1e-6

F32 = mybir.dt.float32
BF16 = mybir.dt.bfloat16

CQ_TILES = Q_LORA // 128  # 12
CKV_TILES = KV_LORA // 128  # 4
HT_TILES = HID // 128  # 32
S_TILES = S // 128  # 16
SQB = 512
NSQB = S // SQB  # 4
EB = 512
NEB = HID // EB  # 8

# phase-1 column blocks, kv+rope first so their collective fires early.
# queue: 0 = sync DGE, 1 = scalar (ACT) DGE
CBLOCKS = [
    (1536, 288, 0),
    (1824, 288, 0),
    (0, 512, 0),
    (512, 512, 0),
    (1024, 512, 1),
]
# feature tiles of the kv+rope latent block: 4x128 (ckv) + 1x64 (rope)
KV_CT = [(0, 128), (128, 128), (256, 128), (384, 128), (512, 64)]  # rel to 1536


# ---------------------------------------------------------------- program
def build_program() -> bass.Bass:
    nc = bacc.Bacc(
        "TRN2",
        target_bir_lowering=False,
        debug=False,
        num_devices=NCORES,
    )

    # hidden states arrive pre-transposed (feature-major) from the host, so
    # no on-device transposes are needed before the a-projection
    hid_d = nc.declare_dram_parameter("hid", [HID, SL], BF16, isOutput=False)
    wa_d = nc.declare_dram_parameter("wa", [HID, C], BF16, isOutput=False)
    wqb_d = nc.declare_dram_parameter("wqb", [Q_LORA, HL * Q_HEAD], BF16, isOutput=False)
    wkvb_d = nc.declare_dram_parameter(
        "wkvb", [KV_LORA, HL * (NOPE + V_DIM)], BF16, isOutput=False
    )
    wo_d = nc.declare_dram_parameter("wo", [HL * V_DIM, HID], BF16, isOutput=False)
    mask_d = nc.declare_dram_parameter("mask", [128, 128], BF16, isOutput=False)
    ones_d = nc.declare_dram_parameter("ones", [128, 1], BF16, isOutput=False)
    onesr_d = nc.declare_dram_parameter("onesr", [1, 128], BF16, isOutput=False)
    out_d = nc.declare_dram_parameter("out", [S, HID], BF16, isOutput=True)

    # collective bounce buffers (internal DRAM). The q latents gather in two
    # pipelined halves so the first half lands before the up-projection
    # needs it.
    NQG = 3  # q gathers: one per a-proj block, fired as each block finishes
    QHT = CQ_TILES // NQG  # 4 tiles per gather
    QH = 128 * QHT  # 512 latent rows per gather
    cc_warm_in = nc.dram_tensor("cc_warm_in", [1, 16], BF16)
    cc_warm_out = nc.dram_tensor("cc_warm_out", [NCORES, 1, 16], BF16, addr_space="Shared")
    cc_in_kv = nc.dram_tensor("cc_in_kv", [CKV_R, SL], BF16)
    cc_out_kv = nc.dram_tensor("cc_out_kv", [NCORES, CKV_R, SL], BF16, addr_space="Shared")
    # the q latents gather UNNORMALIZED (the 1/rms row scale is applied after
    # the up-projection); the last gather carries one extra row with the
    # per-row inv-rms factors
    cc_in_q = [
        nc.dram_tensor(f"cc_in_q{i}", [QH + (1 if i == NQG - 1 else 0), SL], BF16)
        for i in range(NQG)
    ]
    cc_out_q = [
        nc.dram_tensor(
            f"cc_out_q{i}",
            [NCORES, QH + (1 if i == NQG - 1 else 0), SL],
            BF16,
            addr_space="Shared",
        )
        for i in range(NQG)
    ]

    with tile.TileContext(nc, num_cores=NCORES) as tc, ExitStack() as stack:
        # ---------------- small persistent constants
        misc = stack.enter_context(tc.tile_pool(name="misc", bufs=1))
        ident = misc.tile([128, 128], BF16, tag="ident", name="ident")
        make_identity(nc, ident[:])
        ones_sb = misc.tile([128, 1], BF16, tag="ones", name="ones")
        mask_sb = misc.tile([128, 128], BF16, tag="mask", name="mask")
        onesr_sb = misc.tile([1, 128], BF16, tag="onesr", name="onesr")
        eps_sb = misc.tile([128, 1], F32, tag="eps", name="eps")
        nc.gpsimd.memset(eps_sb[:], EPS)

        # tiny warm-up collective: absorbs the CC-stream init barrier +
        # cross-core skew so the kv-latent gather starts with minimal delay
        nc.gpsimd.collective_compute(
            "AllGather",
            mybir.AluOpType.bypass,
            replica_groups=[list(range(NCORES))],
            ins=[cc_warm_in[:].opt()],
            outs=[cc_warm_out[:].opt()],
        )

        # phase-2 weights + kv latents: allocated before phase 1 so their DMAs
        # prefetch during phase-1 compute (the dma_start calls are emitted
        # inside phase 1, after the hidden-state loads, so the scalar queue
        # delivers hid first).
        wkvb_pool = stack.enter_context(tc.tile_pool(name="wkvb", bufs=1))
        wkvb_sb = [
            wkvb_pool.tile(
                [128, HL * (NOPE + V_DIM)], BF16, tag=f"wkvb{kt}", name=f"wkvb{kt}"
            )
            for kt in range(CKV_TILES)
        ]
        wqb_pool = stack.enter_context(tc.tile_pool(name="wqb", bufs=1))
        wqb_sb = [
            wqb_pool.tile([128, HL * Q_HEAD], BF16, tag=f"wqb{kt}", name=f"wqb{kt}")
            for kt in range(CQ_TILES)
        ]
        latkv = stack.enter_context(tc.tile_pool(name="latkv", bufs=1))
        latkv_sb = [
            latkv.tile([w, S], BF16, tag=f"latkv{i}", name=f"latkv{i}")
            for i, (_, w) in enumerate(KV_CT)
        ]
        kpeT = latkv_sb[-1]  # [64, S]

        # ---------------- phase 1: a-projection on local rows
        with ExitStack() as p1:
            wa_pool = p1.enter_context(tc.tile_pool(name="wa", bufs=2))
            p1_pool = p1.enter_context(tc.tile_pool(name="p1", bufs=1))
            hidT = [
                p1_pool.tile([128, SL], BF16, tag=f"hidT{ht}", name=f"hidT{ht}")
                for ht in range(HT_TILES)
            ]
            # feature-major hidden tiles all on the scalar queue (2MB total,
            # streamed in consumption order) so the sync queue's first bytes
            # are the kv-block weights the first matmuls also need
            for ht in range(HT_TILES):
                nc.scalar.dma_start(hidT[ht][:], hid_d[ht * 128 : (ht + 1) * 128, :])
            # small constants + phase-2 up-proj weights behind hid on the
            # scalar queue (needed from ~80us on; transfers finish by ~25us)
            nc.scalar.dma_start(mask_sb[:], mask_d[:])
            nc.scalar.dma_start(ones_sb[:], ones_d[:])
            nc.scalar.dma_start(onesr_sb[:], onesr_d[:])
            for kt in range(CKV_TILES):
                nc.scalar.dma_start(
                    wkvb_sb[kt][:], wkvb_d[kt * 128 : (kt + 1) * 128, :]
                )
            for kt in range(CQ_TILES):
                nc.scalar.dma_start(wqb_sb[kt][:], wqb_d[kt * 128 : (kt + 1) * 128, :])
            lat_sb = [
                p1_pool.tile([128, C], BF16, tag=f"lat{s2}", name=f"lat{s2}")
                for s2 in range(2)
            ]
            stat = p1_pool.tile([128, 12], F32, tag="stat", name="stat")
            # local latents^T staging (feature-major, [*, SL])
            latTq_loc = [
                p1_pool.tile([128, SL], BF16, tag=f"latTq{ct}", name=f"latTq{ct}")
                for ct in range(CQ_TILES)
            ]
            latTkv_loc = [
                p1_pool.tile([w, SL], BF16, tag=f"latTkv{i}", name=f"latTkv{i}")
                for i, (_, w) in enumerate(KV_CT)
            ]

            tps_pool = p1.enter_context(tc.tile_pool(name="tps", bufs=2, space="PSUM"))
            psum1 = p1.enter_context(tc.tile_pool(name="psum1", bufs=5, space="PSUM"))

            def load_block(c0, cw, q):
                wa_t = []
                pfx = "wakv" if c0 >= Q_LORA else "wa"
                eng = nc.sync if q == 0 else nc.scalar
                for ht in range(HT_TILES):
                    t = wa_pool.tile(
                        [128, cw], BF16, tag=f"{pfx}{ht}", name=f"wa{ht}_{c0}"
                    )
                    eng.dma_start(
                        t[:], wa_d[ht * 128 : (ht + 1) * 128, c0 : c0 + cw]
                    )
                    wa_t.append(t)
                return wa_t

            wa_tiles = {(c0, cw): load_block(c0, cw, q) for c0, cw, q in CBLOCKS}

            def fused_block(c0, cw):
                """fused[:, c0:c0+cw] = hidden @ w_qkv_a[:, c0:c0+cw] (both s-tiles)"""
                wa_t = wa_tiles[(c0, cw)]
                for s2 in range(2):
                    pf = psum1.tile([128, cw], F32, tag="pf", name=f"pf{c0}_{s2}")
                    for ht in range(HT_TILES):
                        nc.tensor.matmul(
                            pf[:],
                            hidT[ht][:, s2 * 128 : (s2 + 1) * 128],
                            wa_t[ht][:],
                            start=(ht == 0),
                            stop=(ht == HT_TILES - 1),
                        )
                    nc.scalar.copy(lat_sb[s2][:, c0 : c0 + cw], pf[:])

            def transpose_lat(src_col, w, dst):
                """dst[:, s2*128...] = lat_sb[s2][:, src_col:src_col+w]ᵀ"""
                for s2 in range(2):
                    pt = tps_pool.tile([128, 128], BF16, tag="tps", name="tpsl")
                    nc.tensor.transpose(
                        pt[:w, :], lat_sb[s2][:, src_col : src_col + w], ident[:]
                    )
                    nc.vector.tensor_copy(
                        dst[:, s2 * 128 : (s2 + 1) * 128], pt[:w, :]
                    )

            def rms_square(col0, ch, stat_base):
                """Accumulate sum-of-squares of lat_sb[:, col0+512ch : ...]."""
                for s2 in range(2):
                    sq = psum1.tile([128, 512], F32, tag="pf", name=f"sq{s2}_{ch}")
                    nc.scalar.activation(
                        sq[:],
                        lat_sb[s2][:, col0 + ch * 512 : col0 + (ch + 1) * 512],
                        mybir.ActivationFunctionType.Square,
                        accum_out=stat[:, stat_base + 3 * s2 + ch : stat_base + 3 * s2 + ch + 1],
                    )

            def rms_finalize(col0, ncols, stat_base, scale=True):
                """Compute per-row 1/rms into stat[:, stat_base+3*s2+2]; if
                `scale`, also scale lat_sb[:, col0:col0+ncols] in place."""
                nch = ncols // 512
                for s2 in range(2):
                    sb = stat_base + 3 * s2
                    for ch in range(1, nch):
                        nc.vector.tensor_add(
                            stat[:, sb : sb + 1],
                            stat[:, sb : sb + 1],
                            stat[:, sb + ch : sb + ch + 1],
                        )
                    nc.scalar.activation(
                        stat[:, sb + 1 : sb + 2],
                        stat[:, sb : sb + 1],
                        mybir.ActivationFunctionType.Sqrt,
                        scale=1.0 / ncols,
                        bias=eps_sb[:],
                    )
                    nc.vector.reciprocal(
                        stat[:, sb + 2 : sb + 3],
                        stat[:, sb + 1 : sb + 2],
                    )
                    if scale:
                        nc.scalar.activation(
                            lat_sb[s2][:, col0 : col0 + ncols],
                            lat_sb[s2][:, col0 : col0 + ncols],
                            mybir.ActivationFunctionType.Copy,
                            scale=stat[:, sb + 2 : sb + 3],
                        )

            # ---- kv + rope blocks first
            fused_block(1536, 288)
            fused_block(1824, 288)
            rms_square(Q_LORA, 0, 0)
            rms_finalize(Q_LORA, KV_LORA, 0)
            # staging on the otherwise-idle gpsimd queue so it never waits
            # behind weight streaming on the HW DGE queues
            for i, (rel, w) in enumerate(KV_CT):
                transpose_lat(Q_LORA + rel, w, latTkv_loc[i])
                nc.gpsimd.dma_start(
                    cc_in_kv[rel : rel + w, :], latTkv_loc[i][:]
                )
            nc.gpsimd.collective_compute(
                "AllGather",
                mybir.AluOpType.bypass,
                replica_groups=[list(range(NCORES))],
                ins=[cc_in_kv[:].opt()],
                outs=[cc_out_kv[:].opt()],
            )
            # bring gathered kv latents into SBUF (early, overlaps q blocks),
            # split across both HW queues to halve the landing latency
            cc_kv_view = cc_out_kv[:].rearrange("j c s -> c j s")
            for i, (rel, w) in enumerate(KV_CT):
                eng = nc.scalar if i % 2 == 0 else nc.sync
                eng.dma_start(
                    latkv_sb[i][:].rearrange("c (j s) -> c j s", j=NCORES),
                    cc_kv_view[rel : rel + w],
                )

            # ---- q blocks. Each block is transposed, staged, and gathered
            # UNNORMALIZED as soon as it finishes (the 1/rms row scale is
            # folded into the up-projection's psum->sbuf copies), so the
            # gathers overlap the remaining a-projection instead of waiting
            # for the rms over all 1536 columns.
            def stage_q_block(g, gather):
                for ctg in range(QHT):
                    ct = g * QHT + ctg
                    transpose_lat(ct * 128, 128, latTq_loc[ct])
                    nc.gpsimd.dma_start(
                        cc_in_q[g][ctg * 128 : (ctg + 1) * 128, :],
                        latTq_loc[ct][:],
                    )
                if gather:
                    nc.gpsimd.collective_compute(
                        "AllGather",
                        mybir.AluOpType.bypass,
                        replica_groups=[list(range(NCORES))],
                        ins=[cc_in_q[g][:].opt()],
                        outs=[cc_out_q[g][:].opt()],
                    )

            fused_block(0, 512)
            rms_square(0, 0, 6)
            stage_q_block(0, gather=True)
            fused_block(512, 512)
            rms_square(0, 1, 6)
            stage_q_block(1, gather=True)
            fused_block(1024, 512)
            rms_square(0, 2, 6)
            stage_q_block(2, gather=False)
            rms_finalize(0, Q_LORA, 6, scale=False)

            # inv-rms row rides along in the last gather: stat cols (8, 11)
            # -> bf16 [128, 2] -> transpose -> [2, 128] -> one DRAM row
            rtmp = p1_pool.tile([128, 2], BF16, tag="rtmp", name="rtmp")
            nc.vector.tensor_copy(rtmp[:, 0:1], stat[:, 8:9])
            nc.vector.tensor_copy(rtmp[:, 1:2], stat[:, 11:12])
            rpt = tps_pool.tile([128, 128], BF16, tag="tps", name="rpt")
            nc.tensor.transpose(rpt[:2, :], rtmp[:], ident[:])
            rT = p1_pool.tile([2, 128], BF16, tag="rT", name="rT")
            nc.vector.tensor_copy(rT[:], rpt[:2, :])
            nc.gpsimd.dma_start(
                cc_in_q[2][QH : QH + 1, :].rearrange(
                    "one (p f) -> (one p) f", p=2
                ),
                rT[:],
            )
            nc.gpsimd.collective_compute(
                "AllGather",
                mybir.AluOpType.bypass,
                replica_groups=[list(range(NCORES))],
                ins=[cc_in_q[2][:].opt()],
                outs=[cc_out_q[2][:].opt()],
            )

        # ---------------- phase 2
        kvpool = stack.enter_context(tc.tile_pool(name="kvpool", bufs=1))
        knopeT = [
            kvpool.tile([128, S], BF16, tag=f"knopeT{h}", name=f"knopeT{h}")
            for h in range(HL)
        ]
        v_sb = [
            kvpool.tile([128, HL * V_DIM], BF16, tag=f"v{st}", name=f"v{st}")
            for st in range(S_TILES)
        ]

        with ExitStack() as p2kv:
            # ---- k_nope^T and V up-projections: only depend on the early kv
            # collective, so they run while the q latents are gathered
            pkv_pool = p2kv.enter_context(tc.tile_pool(name="pkv", bufs=4, space="PSUM"))
            for h in range(HL):
                for skb in range(NSQB):
                    pk = pkv_pool.tile([128, SQB], F32, tag="pkv", name="pk")
                    for kt in range(CKV_TILES):
                        nc.tensor.matmul(
                            pk[:],
                            wkvb_sb[kt][
                                :, h * (NOPE + V_DIM) : h * (NOPE + V_DIM) + NOPE
                            ],
                            latkv_sb[kt][:, skb * SQB : (skb + 1) * SQB],
                            start=(kt == 0),
                            stop=(kt == CKV_TILES - 1),
                        )
                    nc.scalar.copy(knopeT[h][:, skb * SQB : (skb + 1) * SQB], pk[:])
            for st in range(S_TILES):
                pv = pkv_pool.tile([128, HL * V_DIM], F32, tag="pkv", name="pv")
                for kt in range(CKV_TILES):
                    rhs = wkvb_sb[kt][:].rearrange("c (h d) -> c h d", h=HL)[:, :, NOPE:]
                    nc.tensor.matmul(
                        pv[:],
                        latkv_sb[kt][:, st * 128 : (st + 1) * 128],
                        rhs,
                        start=(kt == 0),
                        stop=(kt == CKV_TILES - 1),
                    )
                nc.scalar.copy(v_sb[st][:], pv[:])

        qT = stack.enter_context(tc.tile_pool(name="qT", bufs=1))
        qTA = [qT.tile([128, S], BF16, tag=f"qTA{h}", name=f"qTA{h}") for h in range(HL)]
        qTB = [qT.tile([64, S], BF16, tag=f"qTB{h}", name=f"qTB{h}") for h in range(HL)]
        outT_pool = stack.enter_context(tc.tile_pool(name="outT", bufs=1))
        outT = [
            outT_pool.tile([128, S], BF16, tag=f"outT{h}", name=f"outT{h}")
            for h in range(HL)
        ]

        # q^T (scoped: big q-latents released after). wqb column layout (host
        # side): [A(h0)|A(h1)|A(h2)|A(h3)|B(h0)|B(h1)|B(h2)|B(h3)] so the
        # 64-wide rope (B) parts of head pairs pack into M=128 matmuls.
        with ExitStack() as p2q:
            latq = p2q.enter_context(tc.tile_pool(name="latq", bufs=1))
            latq_sb = [
                latq.tile([128, S], BF16, tag=f"latq{ct}", name=f"latq{ct}")
                for ct in range(CQ_TILES)
            ]
            for g in range(NQG):
                cc_q_view = cc_out_q[g][:].rearrange("j c s -> c j s")
                for ctg in range(QHT):
                    ct = g * QHT + ctg
                    eng = nc.scalar if ct % 2 == 0 else nc.sync
                    eng.dma_start(
                        latq_sb[ct][:].rearrange("c (j s) -> c j s", j=NCORES),
                        cc_q_view[ctg * 128 : (ctg + 1) * 128],
                    )
            # inv-rms row -> broadcast tile [128, S] used to scale the
            # up-projected q as it leaves PSUM
            bcr_pool = p2q.enter_context(tc.tile_pool(name="bcr", bufs=1))
            r_sb = bcr_pool.tile([1, S], BF16, tag="rsb", name="rsb")
            nc.scalar.dma_start(
                r_sb[:].rearrange("one (j s) -> one j s", j=NCORES),
                cc_out_q[2][:, QH : QH + 1, :].rearrange("j one s -> one j s"),
            )
            bc_r = bcr_pool.tile([128, S], F32, tag="bcr", name="bcr")
            pq_pool = p2q.enter_context(tc.tile_pool(name="pq", bufs=8, space="PSUM"))
            for blk in range(4):
                bcp = pq_pool.tile([128, 512], F32, tag="pq", name=f"bcp{blk}")
                nc.tensor.matmul(
                    bcp[:],
                    onesr_sb[:],
                    r_sb[:, blk * 512 : (blk + 1) * 512],
                    start=True,
                    stop=True,
                )
                nc.scalar.copy(bc_r[:, blk * 512 : (blk + 1) * 512], bcp[:])

            def qup_pair(colA, copiesA, colB, copiesB):
                """Two M=128 up-proj passes with the kt loop outermost, so
                twice the PE work covers the wait for each gathered latq
                tile."""
                plan = [(colA, copiesA), (colB, copiesB)]
                pqs = {
                    col0: [
                        pq_pool.tile([128, SQB], F32, tag="pq", name=f"pq{col0}_{sqb}")
                        for sqb in range(NSQB)
                    ]
                    for col0, _ in plan
                }
                for kt in range(CQ_TILES):
                    for col0, _ in plan:
                        for sqb in range(NSQB):
                            nc.tensor.matmul(
                                pqs[col0][sqb][:],
                                wqb_sb[kt][:, col0 : col0 + 128],
                                latq_sb[kt][:, sqb * SQB : (sqb + 1) * SQB],
                                start=(kt == 0),
                                stop=(kt == CQ_TILES - 1),
                            )
                for col0, copies in plan:
                    for sqb in range(NSQB):
                        for dst, (r0, r1) in copies:
                            # psum -> sbuf copy fused with the 1/rms row scale
                            nc.vector.tensor_mul(
                                dst[:, sqb * SQB : (sqb + 1) * SQB],
                                pqs[col0][sqb][r0:r1, :],
                                bc_r[r0:r1, sqb * SQB : (sqb + 1) * SQB],
                            )

            # head 0's A and B parts first so attention's first tiles aren't
            # gated on the last psum copies
            qup_pair(
                0 * 128, [(qTA[0], (0, 128))],
                HL * 128, [(qTB[0], (0, 64)), (qTB[1], (64, 128))],
            )
            qup_pair(
                1 * 128, [(qTA[1], (0, 128))],
                HL * 128 + 128, [(qTB[2], (0, 64)), (qTB[3], (64, 128))],
            )
            qup_pair(
                2 * 128, [(qTA[2], (0, 128))],
                3 * 128, [(qTA[3], (0, 128))],
            )

        wo_pool = stack.enter_context(tc.tile_pool(name="wo", bufs=1))
        wo_sb = [
            wo_pool.tile([128, HID], BF16, tag=f"wo{h}", name=f"wo{h}")
            for h in range(HL)
        ]

        # ---------------- attention (causal, block-skipped, diagonal-trimmed)
        # Score tiles: for (h, bq), k-tiles tk = 0..4(bq+1)-1; the 4 diagonal
        # tiles (d = tk-4bq >= 0) only compute their valid column range
        # [128d, 512). exp'd tiles are masked post-exp (x 0/1 bf16 mask) on
        # their boundary 128-chunk, accumulated into `acc` on the DVE; the
        # softmax denominator is ONE [128,1]-ones matmul on acc per (h, bq).
        # Software-pipelined: AV matmuls trail the score tiles by two tiles;
        # the renormalization epilogue trails by one (h, bq) pair.
        with ExitStack() as p2a:
            ps_pool = p2a.enter_context(tc.tile_pool(name="ps", bufs=4, space="PSUM"))
            psum_sum_pool = p2a.enter_context(
                tc.tile_pool(name="psums", bufs=2, space="PSUM")
            )
            psum_o_pool = p2a.enter_context(
                tc.tile_pool(name="psumo", bufs=2, space="PSUM")
            )
            a_pool = p2a.enter_context(tc.tile_pool(name="apool", bufs=6))
            acc_pool = p2a.enter_context(tc.tile_pool(name="accp", bufs=2))
            bc_pool = p2a.enter_context(tc.tile_pool(name="bcpool", bufs=3))

            tile_q = []  # score tiles awaiting their AV matmuls
            ep_q = []  # pairs awaiting the renormalization epilogue

            def drain_tile():
                a, w, h, bq, tk, nk, acc, po = tile_q.pop(0)
                off = SQB - w
                nc.tensor.matmul(
                    po[:, off:SQB],
                    v_sb[tk][:, h * V_DIM : (h + 1) * V_DIM],
                    a[:, :w],
                    start=(tk == 0),
                    stop=(tk == nk - 1),
                    skip_group_check=True,
                )
                if tk == nk - 1:
                    # softmax denominator: one M=1 matmul over the DVE-summed
                    # exp accumulator, then fast reciprocal -> bf16
                    psum = psum_sum_pool.tile([1, SQB], F32, tag="psums", name="psum")
                    nc.tensor.matmul(
                        psum[:], ones_sb[:], acc[:], start=True, stop=True
                    )
                    rs32 = bc_pool.tile([1, SQB], F32, tag="rs32", name="rs32")
                    rs = bc_pool.tile([1, SQB], BF16, tag="rs", name="rs")
                    nc.vector.reciprocal_approx_fast(rs32[:], psum[:])
                    nc.vector.tensor_copy(rs[:], rs32[:])
                    ep_q.append((h, bq, po, rs))

            def drain_epilogue():
                h, bq, po, rs = ep_q.pop(0)
                bc_ps = ps_pool.tile([128, SQB], F32, tag="ps", name="bc_ps")
                nc.tensor.matmul(bc_ps[:], onesr_sb[:], rs[:], start=True, stop=True)
                bc_sb = bc_pool.tile([128, SQB], F32, tag="bc", name="bc_sb")
                nc.scalar.copy(bc_sb[:], bc_ps[:])
                nc.vector.tensor_mul(
                    outT[h][:, bq * SQB : (bq + 1) * SQB], po[:], bc_sb[:]
                )

            for h in range(HL):
                for bq in range(NSQB):
                    nk = 4 * (bq + 1)
                    acc = acc_pool.tile([128, SQB], BF16, tag="acc", name="acc")
                    po = psum_o_pool.tile([128, SQB], F32, tag="psumo", name="po")
                    for tk in range(nk):
                        d = tk - 4 * bq
                        w = SQB if d < 0 else SQB - 128 * d  # valid cols
                        off = SQB - w  # offset inside the sq block
                        ps = ps_pool.tile([128, SQB], F32, tag="ps", name="ps")
                        nc.tensor.matmul(
                            ps[:, :w],
                            knopeT[h][:, tk * 128 : (tk + 1) * 128],
                            qTA[h][:, bq * SQB + off : (bq + 1) * SQB],
                            start=True,
                            stop=False,
                        )
                        nc.tensor.matmul(
                            ps[:, :w],
                            kpeT[:, tk * 128 : (tk + 1) * 128],
                            qTB[h][:, bq * SQB + off : (bq + 1) * SQB],
                            start=False,
                            stop=True,
                        )
                        a = a_pool.tile([128, SQB], BF16, tag="a", name="a")
                        nc.scalar.activation(
                            a[:, :w], ps[:, :w], mybir.ActivationFunctionType.Exp
                        )
                        if d >= 0:
                            # boundary chunk: zero the causally-invalid part
                            nc.vector.tensor_mul(
                                a[:, :128], a[:, :128], mask_sb[:]
                            )
                        if tk == 0:
                            nc.vector.tensor_copy(acc[:], a[:])
                        else:
                            nc.vector.tensor_add(
                                acc[:, off:SQB], acc[:, off:SQB], a[:, :w]
                            )
                        tile_q.append((a, w, h, bq, tk, nk, acc, po))
                        while len(tile_q) > 2:
                            drain_tile()
                        while len(ep_q) > 1:
                            drain_epilogue()
                if h == 0:
                    # o_proj weights stream during attention, after the q
                    # gathers are off the wire
                    for hh in range(HL):
                        nc.scalar.dma_start(
                            wo_sb[hh][:], wo_d[hh * 128 : (hh + 1) * 128, :]
                        )
            while tile_q:
                drain_tile()
            while ep_q:
                drain_epilogue()

        # ---------------- o_proj (partial: summed across cores on host)
        with ExitStack() as p2o:
            pe_pool = p2o.enter_context(tc.tile_pool(name="pe", bufs=4, space="PSUM"))
            stage_pool = p2o.enter_context(tc.tile_pool(name="stage", bufs=4))
            for st in range(S_TILES):
                for half in range(2):
                    stg = stage_pool.tile([128, 4 * EB], BF16, tag="stage", name="stg")
                    for ebl in range(4):
                        eb = half * 4 + ebl
                        pe = pe_pool.tile([128, EB], F32, tag="pe", name="pe")
                        for h in range(HL):
                            nc.tensor.matmul(
                                pe[:],
                                outT[h][:, st * 128 : (st + 1) * 128],
                                wo_sb[h][:, eb * EB : (eb + 1) * EB],
                                start=(h == 0),
                                stop=(h == HL - 1),
                            )
                        if ebl % 2 == 0:
                            nc.vector.tensor_copy(
                                stg[:, ebl * EB : (ebl + 1) * EB], pe[:]
                            )
                        else:
                            nc.scalar.copy(
                                stg[:, ebl * EB : (ebl + 1) * EB], pe[:]
                            )
                    nc.sync.dma_start(
                        out_d[
                            st * 128 : (st + 1) * 128,
                            half * 4 * EB : (half + 1) * 4 * EB,
                        ],
                        stg[:],
                    )

    nc.compile()
    return nc


_PROGRAM_CACHE = {}


def _get_program() -> bass.Bass:
    if "nc" not in _PROGRAM_CACHE:
        _PROGRAM_CACHE["nc"] = build_program()
    return _PROGRAM_CACHE["nc"]


def _make_mask() -> np.ndarray:
    # 0/1 triangular boundary mask for diagonal score tiles, applied post-exp:
    # within the boundary 128-chunk, valid iff local col j >= partition p.
    p = np.arange(128)[:, None]
    j = np.arange(128)[None, :]
    return np.where(j >= p, 1.0, 0.0).astype(ml_dtypes.bfloat16)


def prepare_inputs(
    hidden_states, w_qkv_a, q_a_gamma, w_q_b, kv_a_gamma, w_kv_b, w_o, b_o
):
    """Host-side prep: fold gammas + attention scale into B weights, cast to
    bf16, slice per core."""
    bf = ml_dtypes.bfloat16
    hs = np.asarray(hidden_states, np.float32).reshape(S, HID)
    scale = float(Q_HEAD) ** -0.5
    wqb_eff = (
        np.asarray(w_q_b, np.float32)
        * np.asarray(q_a_gamma, np.float32)[:, None]
        * scale
    )
    wkvb_eff = (
        np.asarray(w_kv_b, np.float32) * np.asarray(kv_a_gamma, np.float32)[:, None]
    )
    wa_bf = np.asarray(w_qkv_a, np.float32).astype(bf)
    hs_bf = hs.astype(bf)
    mask = _make_mask()
    ones = np.ones((128, 1), bf)
    onesr = np.ones((1, 128), bf)

    wqb_r = wqb_eff.reshape(Q_LORA, H, Q_HEAD)
    wkvb_r = wkvb_eff.reshape(KV_LORA, H, NOPE + V_DIM)
    wo_r = np.asarray(w_o, np.float32).reshape(H, V_DIM, HID)

    in_maps = []
    for c in range(NCORES):
        # feature-major (transposed) slice: [HID, SL]
        hsl = np.ascontiguousarray(hs_bf[c * SL : (c + 1) * SL].T)
        wqb_c = wqb_r[:, c * HL : (c + 1) * HL]  # [Q_LORA, HL, 192]
        # pack columns: A parts (128 each) for all heads, then B parts (64)
        wqb_packed = np.concatenate(
            [wqb_c[:, h, :NOPE] for h in range(HL)]
            + [wqb_c[:, h, NOPE:] for h in range(HL)],
            axis=1,
        )
        wqb_cc = np.ascontiguousarray(wqb_packed.astype(bf))
        wkvb_c = np.ascontiguousarray(
            wkvb_r[:, c * HL : (c + 1) * HL]
            .reshape(KV_LORA, HL * (NOPE + V_DIM))
            .astype(bf)
        )
        wo_c = np.ascontiguousarray(
            wo_r[c * HL : (c + 1) * HL].reshape(HL * V_DIM, HID).astype(bf)
        )
        in_maps.append(
            {
                "hid": hsl,
                "wa": wa_bf,
                "wqb": wqb_cc,
                "wkvb": wkvb_c,
                "wo": wo_c,
                "mask": mask,
                "ones": ones,
                "onesr": onesr,
            }
        )
    return in_maps


def kernel(**inputs) -> np.ndarray:
    in_maps = prepare_inputs(**inputs)
    nc = _get_program()
    res = run_bass_kernel_spmd(nc, in_maps, list(range(NCORES)))
    out = np.zeros((S, HID), np.float32)
    for r in res.results:
        out += np.asarray(r["out"], np.float32)
    out = out + np.asarray(inputs["b_o"], np.float32)[None, :]
    return out.reshape(1, S, HID)


# revision 46
# speedup vs baseline: 1.0152x; 1.0152x over previous
"""
MLA attention (DeepSeek-style) on 8 TRN2 NeuronCores.

Sharding:
  phase 1 (LoRA-A projection + RMSNorm): sharded over sequence (256 rows/core),
    result transposed to feature-major and AllGathered (bf16 latents).
    The kv+rope latent columns are computed first and gathered in an early
    collective that overlaps the rest of phase 1; the q latents follow.
  phase 2 (q/kv up-proj, attention, o_proj): sharded over heads (4 heads/core),
    w_o input-dim sharded; partial outputs summed on the host (the all-reduce).

All heavy matmuls run in bf16 with fp32 PSUM accumulation.
Everything feature-major ("X^T" layout [feature, seq]) in phase 2 so no big
transposes are needed.

v2 changes vs baseline:
  - DMA queue spreading: weight streaming on sync+vector, collective staging
    on gpsimd (so gathers fire ~100us earlier), gathered latents pulled in on
    scalar+sync.
  - softmax row-sums via DVE accumulation of exp tiles + ONE M=1 matmul per
    (head, sq-block) instead of one per score tile (-74K PE columns).
  - the 1/rowsum broadcast matmul runs in bf16 (was fp32 = 4x column rate),
    reciprocal via the fast DVE approximation.
  - causal diagonal tiles trimmed to their valid column range (scores, exp,
    AV); the boundary 128-wide chunk is masked post-exp with a 0/1 bf16 mask.
  - rope-part q up-projection packed across head pairs (M=128 instead of 64).
  - partial outputs written bf16 (halves the output stream).
"""

import os
import sys
from contextlib import ExitStack

import numpy as np

for _p in ("/opt/trn_rl_repo", "/root/.axon_site/_ro/trn_rl_repo"):
    if os.path.isdir(_p) and _p not in sys.path:
        sys.path.insert(0, _p)

import ml_dtypes  # noqa: E402

import concourse.bacc as bacc  # noqa: E402
import concourse.bass as bass  # noqa: E402
import concourse.mybir as mybir  # noqa: E402
import concourse.tile as tile  # noqa: E402
from concourse.bass_utils import run_bass_kernel_spmd  # noqa: E402
from concourse.masks import make_identity  # noqa: E402

# ---------------------------------------------------------------- constants
NCORES = 8
S = 2048
SL = S // NCORES  # 256 local rows in phase 1
HID = 4096
Q_LORA = 1536
KV_LORA = 512
ROPE = 64
C = Q_LORA + KV_LORA + ROPE  # 2112
CKV_R = KV_LORA + ROPE  # 576 kv+rope latent cols
NOPE = 128
V_DIM = 128
H = 32
HL = H // NCORES  # 4 local heads
Q_HEAD = NOPE + ROPE  # 192
EPS = 1e-6

F32 = mybir.dt.float32
BF16 = mybir.dt.bfloat16

CQ_TILES = Q_LORA // 128  # 12
CKV_TILES = KV_LORA // 128  # 4
HT_TILES = HID // 128  # 32
S_TILES = S // 128  # 16
SQB = 512
NSQB = S // SQB  # 4
EB = 512
NEB = HID // EB  # 8

# phase-1 column blocks, kv+rope first so their collective fires early.
# All weight streaming lives on the sync DGE queue: its trigger instructions
# carry buffer-reuse waits paced by PE progress, which must never sit in
# front of the scalar (ACT) queue's compute.
CBLOCKS = [
    (1536, 288, 0),
    (1824, 288, 0),
    (0, 512, 0),
    (512, 512, 0),
    (1024, 512, 0),
]
# feature tiles of the kv+rope latent block: 4x128 (ckv) + 1x64 (rope)
KV_CT = [(0, 128), (128, 128), (256, 128), (384, 128), (512, 64)]  # rel to 1536


# ---------------------------------------------------------------- program
def build_program() -> bass.Bass:
    nc = bacc.Bacc(
        "TRN2",
        target_bir_lowering=False,
        debug=False,
        num_devices=NCORES,
    )

    # hidden states arrive pre-transposed (feature-major) from the host, so
    # no on-device transposes are needed before the a-projection
    hid_d = nc.declare_dram_parameter("hid", [HID, SL], BF16, isOutput=False)
    wa_d = nc.declare_dram_parameter("wa", [HID, C], BF16, isOutput=False)
    wqb_d = nc.declare_dram_parameter("wqb", [Q_LORA, HL * Q_HEAD], BF16, isOutput=False)
    wkvb_d = nc.declare_dram_parameter(
        "wkvb", [KV_LORA, HL * (NOPE + V_DIM)], BF16, isOutput=False
    )
    wo_d = nc.declare_dram_parameter("wo", [HL * V_DIM, HID], BF16, isOutput=False)
    mask_d = nc.declare_dram_parameter("mask", [128, 128], BF16, isOutput=False)
    ones_d = nc.declare_dram_parameter("ones", [128, 1], BF16, isOutput=False)
    onesr_d = nc.declare_dram_parameter("onesr", [1, 128], BF16, isOutput=False)
    out_d = nc.declare_dram_parameter("out", [S, HID], BF16, isOutput=True)

    # collective bounce buffers (internal DRAM). The q latents gather in two
    # pipelined halves so the first half lands before the up-projection
    # needs it.
    NQG = 3  # q gathers: one per a-proj block, fired as each block finishes
    QHT = CQ_TILES // NQG  # 4 tiles per gather
    QH = 128 * QHT  # 512 latent rows per gather
    cc_warm_in = nc.dram_tensor("cc_warm_in", [1, 16], BF16)
    cc_warm_out = nc.dram_tensor("cc_warm_out", [NCORES, 1, 16], BF16, addr_space="Shared")
    cc_in_kv = nc.dram_tensor("cc_in_kv", [CKV_R, SL], BF16)
    cc_out_kv = nc.dram_tensor("cc_out_kv", [NCORES, CKV_R, SL], BF16, addr_space="Shared")
    # the q latents gather UNNORMALIZED (the 1/rms row scale is applied after
    # the up-projection); the last gather carries one extra row with the
    # per-row inv-rms factors
    cc_in_q = [
        nc.dram_tensor(f"cc_in_q{i}", [QH + (1 if i == NQG - 1 else 0), SL], BF16)
        for i in range(NQG)
    ]
    cc_out_q = [
        nc.dram_tensor(
            f"cc_out_q{i}",
            [NCORES, QH + (1 if i == NQG - 1 else 0), SL],
            BF16,
            addr_space="Shared",
        )
        for i in range(NQG)
    ]

    with tile.TileContext(nc, num_cores=NCORES) as tc, ExitStack() as stack:
        # ---------------- small persistent constants
        misc = stack.enter_context(tc.tile_pool(name="misc", bufs=1))
        ident = misc.tile([128, 128], BF16, tag="ident", name="ident")
        make_identity(nc, ident[:])
        ones_sb = misc.tile([128, 1], BF16, tag="ones", name="ones")
        mask_sb = misc.tile([128, 128], BF16, tag="mask", name="mask")
        onesr_sb = misc.tile([1, 128], BF16, tag="onesr", name="onesr")
        eps_sb = misc.tile([128, 1], F32, tag="eps", name="eps")
        nc.gpsimd.memset(eps_sb[:], EPS)

        # tiny warm-up collective: absorbs the CC-stream init barrier +
        # cross-core skew so the kv-latent gather starts with minimal delay
        nc.gpsimd.collective_compute(
            "AllGather",
            mybir.AluOpType.bypass,
            replica_groups=[list(range(NCORES))],
            ins=[cc_warm_in[:].opt()],
            outs=[cc_warm_out[:].opt()],
        )

        # phase-2 weights + kv latents: allocated before phase 1 so their DMAs
        # prefetch during phase-1 compute (the dma_start calls are emitted
        # inside phase 1, after the hidden-state loads, so the scalar queue
        # delivers hid first).
        wkvb_pool = stack.enter_context(tc.tile_pool(name="wkvb", bufs=1))
        wkvb_sb = [
            wkvb_pool.tile(
                [128, HL * (NOPE + V_DIM)], BF16, tag=f"wkvb{kt}", name=f"wkvb{kt}"
            )
            for kt in range(CKV_TILES)
        ]
        wqb_pool = stack.enter_context(tc.tile_pool(name="wqb", bufs=1))
        wqb_sb = [
            wqb_pool.tile([128, HL * Q_HEAD], BF16, tag=f"wqb{kt}", name=f"wqb{kt}")
            for kt in range(CQ_TILES)
        ]
        latkv = stack.enter_context(tc.tile_pool(name="latkv", bufs=1))
        latkv_sb = [
            latkv.tile([w, S], BF16, tag=f"latkv{i}", name=f"latkv{i}")
            for i, (_, w) in enumerate(KV_CT)
        ]
        kpeT = latkv_sb[-1]  # [64, S]

        # ---------------- phase 1: a-projection on local rows
        with ExitStack() as p1:
            wa_pool = p1.enter_context(tc.tile_pool(name="wa", bufs=2))
            p1_pool = p1.enter_context(tc.tile_pool(name="p1", bufs=1))
            hidT = [
                p1_pool.tile([128, SL], BF16, tag=f"hidT{ht}", name=f"hidT{ht}")
                for ht in range(HT_TILES)
            ]
            # feature-major hidden tiles all on the scalar queue (2MB total,
            # streamed in consumption order) so the sync queue's first bytes
            # are the kv-block weights the first matmuls also need
            for ht in range(HT_TILES):
                nc.scalar.dma_start(hidT[ht][:], hid_d[ht * 128 : (ht + 1) * 128, :])
            # small constants + phase-2 up-proj weights on the gpsimd queue:
            # ungated triggers, done well before the kv staging needs it
            nc.gpsimd.dma_start(mask_sb[:], mask_d[:])
            nc.gpsimd.dma_start(ones_sb[:], ones_d[:])
            nc.gpsimd.dma_start(onesr_sb[:], onesr_d[:])
            for kt in range(CKV_TILES):
                nc.gpsimd.dma_start(
                    wkvb_sb[kt][:], wkvb_d[kt * 128 : (kt + 1) * 128, :]
                )
            for kt in range(CQ_TILES):
                nc.gpsimd.dma_start(wqb_sb[kt][:], wqb_d[kt * 128 : (kt + 1) * 128, :])
            lat_sb = [
                p1_pool.tile([128, C], BF16, tag=f"lat{s2}", name=f"lat{s2}")
                for s2 in range(2)
            ]
            stat = p1_pool.tile([128, 12], F32, tag="stat", name="stat")
            # local latents^T staging (feature-major, [*, SL])
            latTq_loc = [
                p1_pool.tile([128, SL], BF16, tag=f"latTq{ct}", name=f"latTq{ct}")
                for ct in range(CQ_TILES)
            ]
            latTkv_loc = [
                p1_pool.tile([w, SL], BF16, tag=f"latTkv{i}", name=f"latTkv{i}")
                for i, (_, w) in enumerate(KV_CT)
            ]

            tps_pool = p1.enter_context(tc.tile_pool(name="tps", bufs=2, space="PSUM"))
            psum1 = p1.enter_context(tc.tile_pool(name="psum1", bufs=5, space="PSUM"))

            def load_block(c0, cw, q):
                wa_t = []
                pfx = "wakv" if c0 >= Q_LORA else "wa"
                eng = nc.sync if q == 0 else nc.scalar
                for ht in range(HT_TILES):
                    t = wa_pool.tile(
                        [128, cw], BF16, tag=f"{pfx}{ht}", name=f"wa{ht}_{c0}"
                    )
                    eng.dma_start(
                        t[:], wa_d[ht * 128 : (ht + 1) * 128, c0 : c0 + cw]
                    )
                    wa_t.append(t)
                return wa_t

            wa_tiles = {(c0, cw): load_block(c0, cw, q) for c0, cw, q in CBLOCKS}

            def fused_block(c0, cw):
                """fused[:, c0:c0+cw] = hidden @ w_qkv_a[:, c0:c0+cw] (both s-tiles)"""
                wa_t = wa_tiles[(c0, cw)]
                for s2 in range(2):
                    pf = psum1.tile([128, cw], F32, tag="pf", name=f"pf{c0}_{s2}")
                    for ht in range(HT_TILES):
                        nc.tensor.matmul(
                            pf[:],
                            hidT[ht][:, s2 * 128 : (s2 + 1) * 128],
                            wa_t[ht][:],
                            start=(ht == 0),
                            stop=(ht == HT_TILES - 1),
                        )
                    nc.scalar.copy(lat_sb[s2][:, c0 : c0 + cw], pf[:])

            def transpose_lat(src_col, w, dst):
                """dst[:, s2*128...] = lat_sb[s2][:, src_col:src_col+w]ᵀ"""
                for s2 in range(2):
                    pt = tps_pool.tile([128, 128], BF16, tag="tps", name="tpsl")
                    nc.tensor.transpose(
                        pt[:w, :], lat_sb[s2][:, src_col : src_col + w], ident[:]
                    )
                    nc.vector.tensor_copy(
                        dst[:, s2 * 128 : (s2 + 1) * 128], pt[:w, :]
                    )

            def rms_square(col0, ch, stat_base):
                """Accumulate sum-of-squares of lat_sb[:, col0+512ch : ...]."""
                for s2 in range(2):
                    sq = psum1.tile([128, 512], F32, tag="pf", name=f"sq{s2}_{ch}")
                    nc.scalar.activation(
                        sq[:],
                        lat_sb[s2][:, col0 + ch * 512 : col0 + (ch + 1) * 512],
                        mybir.ActivationFunctionType.Square,
                        accum_out=stat[:, stat_base + 3 * s2 + ch : stat_base + 3 * s2 + ch + 1],
                    )

            def rms_finalize(col0, ncols, stat_base, scale=True):
                """Compute per-row 1/rms into stat[:, stat_base+3*s2+2]; if
                `scale`, also scale lat_sb[:, col0:col0+ncols] in place."""
                nch = ncols // 512
                for s2 in range(2):
                    sb = stat_base + 3 * s2
                    for ch in range(1, nch):
                        nc.vector.tensor_add(
                            stat[:, sb : sb + 1],
                            stat[:, sb : sb + 1],
                            stat[:, sb + ch : sb + ch + 1],
                        )
                    nc.scalar.activation(
                        stat[:, sb + 1 : sb + 2],
                        stat[:, sb : sb + 1],
                        mybir.ActivationFunctionType.Sqrt,
                        scale=1.0 / ncols,
                        bias=eps_sb[:],
                    )
                    nc.vector.reciprocal(
                        stat[:, sb + 2 : sb + 3],
                        stat[:, sb + 1 : sb + 2],
                    )
                    if scale:
                        nc.scalar.activation(
                            lat_sb[s2][:, col0 : col0 + ncols],
                            lat_sb[s2][:, col0 : col0 + ncols],
                            mybir.ActivationFunctionType.Copy,
                            scale=stat[:, sb + 2 : sb + 3],
                        )

            # ---- kv + rope blocks first
            fused_block(1536, 288)
            fused_block(1824, 288)
            rms_square(Q_LORA, 0, 0)
            rms_finalize(Q_LORA, KV_LORA, 0)
            # staging on the otherwise-idle gpsimd queue so it never waits
            # behind weight streaming on the HW DGE queues
            for i, (rel, w) in enumerate(KV_CT):
                transpose_lat(Q_LORA + rel, w, latTkv_loc[i])
                nc.gpsimd.dma_start(
                    cc_in_kv[rel : rel + w, :], latTkv_loc[i][:]
                )
            nc.gpsimd.collective_compute(
                "AllGather",
                mybir.AluOpType.bypass,
                replica_groups=[list(range(NCORES))],
                ins=[cc_in_kv[:].opt()],
                outs=[cc_out_kv[:].opt()],
            )
            # bring gathered kv latents into SBUF (early, overlaps q blocks).
            # Sync queue only: these triggers wait on the collective, and on
            # the scalar queue that wait would block phase-1 ACT compute.
            cc_kv_view = cc_out_kv[:].rearrange("j c s -> c j s")
            for i, (rel, w) in enumerate(KV_CT):
                nc.sync.dma_start(
                    latkv_sb[i][:].rearrange("c (j s) -> c j s", j=NCORES),
                    cc_kv_view[rel : rel + w],
                )

            # ---- q blocks. Each block is transposed, staged, and gathered
            # UNNORMALIZED as soon as it finishes (the 1/rms row scale is
            # folded into the up-projection's psum->sbuf copies), so the
            # gathers overlap the remaining a-projection instead of waiting
            # for the rms over all 1536 columns.
            def stage_q_block(g, gather):
                for ctg in range(QHT):
                    ct = g * QHT + ctg
                    transpose_lat(ct * 128, 128, latTq_loc[ct])
                    nc.gpsimd.dma_start(
                        cc_in_q[g][ctg * 128 : (ctg + 1) * 128, :],
                        latTq_loc[ct][:],
                    )
                if gather:
                    nc.gpsimd.collective_compute(
                        "AllGather",
                        mybir.AluOpType.bypass,
                        replica_groups=[list(range(NCORES))],
                        ins=[cc_in_q[g][:].opt()],
                        outs=[cc_out_q[g][:].opt()],
                    )

            fused_block(0, 512)
            rms_square(0, 0, 6)
            stage_q_block(0, gather=True)
            fused_block(512, 512)
            rms_square(0, 1, 6)
            stage_q_block(1, gather=True)
            fused_block(1024, 512)
            rms_square(0, 2, 6)
            stage_q_block(2, gather=False)
            rms_finalize(0, Q_LORA, 6, scale=False)

            # inv-rms row rides along in the last gather: stat cols (8, 11)
            # -> bf16 [128, 2] -> transpose -> [2, 128] -> one DRAM row
            rtmp = p1_pool.tile([128, 2], BF16, tag="rtmp", name="rtmp")
            nc.vector.tensor_copy(rtmp[:, 0:1], stat[:, 8:9])
            nc.vector.tensor_copy(rtmp[:, 1:2], stat[:, 11:12])
            rpt = tps_pool.tile([128, 128], BF16, tag="tps", name="rpt")
            nc.tensor.transpose(rpt[:2, :], rtmp[:], ident[:])
            rT = p1_pool.tile([2, 128], BF16, tag="rT", name="rT")
            nc.vector.tensor_copy(rT[:], rpt[:2, :])
            nc.gpsimd.dma_start(
                cc_in_q[2][QH : QH + 1, :].rearrange(
                    "one (p f) -> (one p) f", p=2
                ),
                rT[:],
            )
            nc.gpsimd.collective_compute(
                "AllGather",
                mybir.AluOpType.bypass,
                replica_groups=[list(range(NCORES))],
                ins=[cc_in_q[2][:].opt()],
                outs=[cc_out_q[2][:].opt()],
            )

        # ---------------- phase 2
        kvpool = stack.enter_context(tc.tile_pool(name="kvpool", bufs=1))
        knopeT = [
            kvpool.tile([128, S], BF16, tag=f"knopeT{h}", name=f"knopeT{h}")
            for h in range(HL)
        ]
        v_sb = [
            kvpool.tile([128, HL * V_DIM], BF16, tag=f"v{st}", name=f"v{st}")
            for st in range(S_TILES)
        ]

        with ExitStack() as p2kv:
            # ---- k_nope^T and V up-projections: only depend on the early kv
            # collective, so they run while the q latents are gathered
            pkv_pool = p2kv.enter_context(tc.tile_pool(name="pkv", bufs=4, space="PSUM"))
            for h in range(HL):
                for skb in range(NSQB):
                    pk = pkv_pool.tile([128, SQB], F32, tag="pkv", name="pk")
                    for kt in range(CKV_TILES):
                        nc.tensor.matmul(
                            pk[:],
                            wkvb_sb[kt][
                                :, h * (NOPE + V_DIM) : h * (NOPE + V_DIM) + NOPE
                            ],
                            latkv_sb[kt][:, skb * SQB : (skb + 1) * SQB],
                            start=(kt == 0),
                            stop=(kt == CKV_TILES - 1),
                        )
                    nc.scalar.copy(knopeT[h][:, skb * SQB : (skb + 1) * SQB], pk[:])
            for st in range(S_TILES):
                pv = pkv_pool.tile([128, HL * V_DIM], F32, tag="pkv", name="pv")
                for kt in range(CKV_TILES):
                    rhs = wkvb_sb[kt][:].rearrange("c (h d) -> c h d", h=HL)[:, :, NOPE:]
                    nc.tensor.matmul(
                        pv[:],
                        latkv_sb[kt][:, st * 128 : (st + 1) * 128],
                        rhs,
                        start=(kt == 0),
                        stop=(kt == CKV_TILES - 1),
                    )
                nc.scalar.copy(v_sb[st][:], pv[:])

        qT = stack.enter_context(tc.tile_pool(name="qT", bufs=1))
        qTA = [qT.tile([128, S], BF16, tag=f"qTA{h}", name=f"qTA{h}") for h in range(HL)]
        qTB = [qT.tile([64, S], BF16, tag=f"qTB{h}", name=f"qTB{h}") for h in range(HL)]
        outT_pool = stack.enter_context(tc.tile_pool(name="outT", bufs=1))
        outT = [
            outT_pool.tile([128, S], BF16, tag=f"outT{h}", name=f"outT{h}")
            for h in range(HL)
        ]

        # q^T (scoped: big q-latents released after). wqb column layout (host
        # side): [A(h0)|A(h1)|A(h2)|A(h3)|B(h0)|B(h1)|B(h2)|B(h3)] so the
        # 64-wide rope (B) parts of head pairs pack into M=128 matmuls.
        with ExitStack() as p2q:
            latq = p2q.enter_context(tc.tile_pool(name="latq", bufs=1))
            latq_sb = [
                latq.tile([128, S], BF16, tag=f"latq{ct}", name=f"latq{ct}")
                for ct in range(CQ_TILES)
            ]
            for g in range(NQG):
                cc_q_view = cc_out_q[g][:].rearrange("j c s -> c j s")
                for ctg in range(QHT):
                    ct = g * QHT + ctg
                    eng = nc.scalar if ct % 2 == 0 else nc.sync
                    eng.dma_start(
                        latq_sb[ct][:].rearrange("c (j s) -> c j s", j=NCORES),
                        cc_q_view[ctg * 128 : (ctg + 1) * 128],
                    )
            # inv-rms row -> broadcast tile [128, S] used to scale the
            # up-projected q as it leaves PSUM
            bcr_pool = p2q.enter_context(tc.tile_pool(name="bcr", bufs=1))
            r_sb = bcr_pool.tile([1, S], BF16, tag="rsb", name="rsb")
            nc.scalar.dma_start(
                r_sb[:].rearrange("one (j s) -> one j s", j=NCORES),
                cc_out_q[2][:, QH : QH + 1, :].rearrange("j one s -> one j s"),
            )
            bc_r = bcr_pool.tile([128, S], F32, tag="bcr", name="bcr")
            pq_pool = p2q.enter_context(tc.tile_pool(name="pq", bufs=8, space="PSUM"))
            for blk in range(4):
                bcp = pq_pool.tile([128, 512], F32, tag="pq", name=f"bcp{blk}")
                nc.tensor.matmul(
                    bcp[:],
                    onesr_sb[:],
                    r_sb[:, blk * 512 : (blk + 1) * 512],
                    start=True,
                    stop=True,
                )
                nc.scalar.copy(bc_r[:, blk * 512 : (blk + 1) * 512], bcp[:])

            def qup_pair(colA, copiesA, colB, copiesB):
                """Two M=128 up-proj passes with the kt loop outermost, so
                twice the PE work covers the wait for each gathered latq
                tile."""
                plan = [(colA, copiesA), (colB, copiesB)]
                pqs = {
                    col0: [
                        pq_pool.tile([128, SQB], F32, tag="pq", name=f"pq{col0}_{sqb}")
                        for sqb in range(NSQB)
                    ]
                    for col0, _ in plan
                }
                for kt in range(CQ_TILES):
                    for col0, _ in plan:
                        for sqb in range(NSQB):
                            nc.tensor.matmul(
                                pqs[col0][sqb][:],
                                wqb_sb[kt][:, col0 : col0 + 128],
                                latq_sb[kt][:, sqb * SQB : (sqb + 1) * SQB],
                                start=(kt == 0),
                                stop=(kt == CQ_TILES - 1),
                            )
                for col0, copies in plan:
                    for sqb in range(NSQB):
                        for dst, (r0, r1) in copies:
                            # psum -> sbuf copy fused with the 1/rms row scale
                            nc.vector.tensor_mul(
                                dst[:, sqb * SQB : (sqb + 1) * SQB],
                                pqs[col0][sqb][r0:r1, :],
                                bc_r[r0:r1, sqb * SQB : (sqb + 1) * SQB],
                            )

            # head 0's A and B parts first so attention's first tiles aren't
            # gated on the last psum copies
            qup_pair(
                0 * 128, [(qTA[0], (0, 128))],
                HL * 128, [(qTB[0], (0, 64)), (qTB[1], (64, 128))],
            )
            qup_pair(
                1 * 128, [(qTA[1], (0, 128))],
                HL * 128 + 128, [(qTB[2], (0, 64)), (qTB[3], (64, 128))],
            )
            qup_pair(
                2 * 128, [(qTA[2], (0, 128))],
                3 * 128, [(qTA[3], (0, 128))],
            )

        wo_pool = stack.enter_context(tc.tile_pool(name="wo", bufs=1))
        wo_sb = [
            wo_pool.tile([128, HID], BF16, tag=f"wo{h}", name=f"wo{h}")
            for h in range(HL)
        ]

        # ---------------- attention (causal, block-skipped, diagonal-trimmed)
        # Score tiles: for (h, bq), k-tiles tk = 0..4(bq+1)-1; the 4 diagonal
        # tiles (d = tk-4bq >= 0) only compute their valid column range
        # [128d, 512). exp'd tiles are masked post-exp (x 0/1 bf16 mask) on
        # their boundary 128-chunk, accumulated into `acc` on the DVE; the
        # softmax denominator is ONE [128,1]-ones matmul on acc per (h, bq).
        # Software-pipelined: AV matmuls trail the score tiles by two tiles;
        # the renormalization epilogue trails by one (h, bq) pair.
        with ExitStack() as p2a:
            ps_pool = p2a.enter_context(tc.tile_pool(name="ps", bufs=4, space="PSUM"))
            psum_sum_pool = p2a.enter_context(
                tc.tile_pool(name="psums", bufs=2, space="PSUM")
            )
            psum_o_pool = p2a.enter_context(
                tc.tile_pool(name="psumo", bufs=2, space="PSUM")
            )
            a_pool = p2a.enter_context(tc.tile_pool(name="apool", bufs=6))
            acc_pool = p2a.enter_context(tc.tile_pool(name="accp", bufs=2))
            bc_pool = p2a.enter_context(tc.tile_pool(name="bcpool", bufs=3))

            tile_q = []  # score tiles awaiting their AV matmuls
            ep_q = []  # pairs awaiting the renormalization epilogue

            def drain_tile():
                a, w, h, bq, tk, nk, acc, po = tile_q.pop(0)
                off = SQB - w
                nc.tensor.matmul(
                    po[:, off:SQB],
                    v_sb[tk][:, h * V_DIM : (h + 1) * V_DIM],
                    a[:, :w],
                    start=(tk == 0),
                    stop=(tk == nk - 1),
                    skip_group_check=True,
                )
                if tk == nk - 1:
                    # softmax denominator: one M=1 matmul over the DVE-summed
                    # exp accumulator, then fast reciprocal -> bf16
                    psum = psum_sum_pool.tile([1, SQB], F32, tag="psums", name="psum")
                    nc.tensor.matmul(
                        psum[:], ones_sb[:], acc[:], start=True, stop=True
                    )
                    rs32 = bc_pool.tile([1, SQB], F32, tag="rs32", name="rs32")
                    rs = bc_pool.tile([1, SQB], BF16, tag="rs", name="rs")
                    nc.vector.reciprocal_approx_fast(rs32[:], psum[:])
                    nc.vector.tensor_copy(rs[:], rs32[:])
                    ep_q.append((h, bq, po, rs))

            def drain_epilogue():
                h, bq, po, rs = ep_q.pop(0)
                bc_ps = ps_pool.tile([128, SQB], F32, tag="ps", name="bc_ps")
                nc.tensor.matmul(bc_ps[:], onesr_sb[:], rs[:], start=True, stop=True)
                bc_sb = bc_pool.tile([128, SQB], F32, tag="bc", name="bc_sb")
                nc.scalar.copy(bc_sb[:], bc_ps[:])
                nc.vector.tensor_mul(
                    outT[h][:, bq * SQB : (bq + 1) * SQB], po[:], bc_sb[:]
                )

            for h in range(HL):
                for bq in range(NSQB):
                    nk = 4 * (bq + 1)
                    acc = acc_pool.tile([128, SQB], BF16, tag="acc", name="acc")
                    po = psum_o_pool.tile([128, SQB], F32, tag="psumo", name="po")
                    for tk in range(nk):
                        d = tk - 4 * bq
                        w = SQB if d < 0 else SQB - 128 * d  # valid cols
                        off = SQB - w  # offset inside the sq block
                        ps = ps_pool.tile([128, SQB], F32, tag="ps", name="ps")
                        nc.tensor.matmul(
                            ps[:, :w],
                            knopeT[h][:, tk * 128 : (tk + 1) * 128],
                            qTA[h][:, bq * SQB + off : (bq + 1) * SQB],
                            start=True,
                            stop=False,
                        )
                        nc.tensor.matmul(
                            ps[:, :w],
                            kpeT[:, tk * 128 : (tk + 1) * 128],
                            qTB[h][:, bq * SQB + off : (bq + 1) * SQB],
                            start=False,
                            stop=True,
                        )
                        a = a_pool.tile([128, SQB], BF16, tag="a", name="a")
                        nc.scalar.activation(
                            a[:, :w], ps[:, :w], mybir.ActivationFunctionType.Exp
                        )
                        if d >= 0:
                            # boundary chunk: zero the causally-invalid part
                            nc.vector.tensor_mul(
                                a[:, :128], a[:, :128], mask_sb[:]
                            )
                        if tk == 0:
                            nc.vector.tensor_copy(acc[:], a[:])
                        else:
                            nc.vector.tensor_add(
                                acc[:, off:SQB], acc[:, off:SQB], a[:, :w]
                            )
                        tile_q.append((a, w, h, bq, tk, nk, acc, po))
                        while len(tile_q) > 2:
                            drain_tile()
                        while len(ep_q) > 1:
                            drain_epilogue()
                if h == 0:
                    # o_proj weights stream during attention, after the q
                    # gathers are off the wire
                    for hh in range(HL):
                        nc.scalar.dma_start(
                            wo_sb[hh][:], wo_d[hh * 128 : (hh + 1) * 128, :]
                        )
            while tile_q:
                drain_tile()
            while ep_q:
                drain_epilogue()

        # ---------------- o_proj (partial: summed across cores on host)
        with ExitStack() as p2o:
            pe_pool = p2o.enter_context(tc.tile_pool(name="pe", bufs=4, space="PSUM"))
            stage_pool = p2o.enter_context(tc.tile_pool(name="stage", bufs=4))
            for st in range(S_TILES):
                for half in range(2):
                    stg = stage_pool.tile([128, 4 * EB], BF16, tag="stage", name="stg")
                    for ebl in range(4):
                        eb = half * 4 + ebl
                        pe = pe_pool.tile([128, EB], F32, tag="pe", name="pe")
                        for h in range(HL):
                            nc.tensor.matmul(
                                pe[:],
                                outT[h][:, st * 128 : (st + 1) * 128],
                                wo_sb[h][:, eb * EB : (eb + 1) * EB],
                                start=(h == 0),
                                stop=(h == HL - 1),
                            )
                        if ebl % 2 == 0:
                            nc.vector.tensor_copy(
                                stg[:, ebl * EB : (ebl + 1) * EB], pe[:]
                            )
                        else:
                            nc.scalar.copy(
                                stg[:, ebl * EB : (ebl + 1) * EB], pe[:]
                            )
                    nc.sync.dma_start(
                        out_d[
                            st * 128 : (st + 1) * 128,
                            half * 4 * EB : (half + 1) * 4 * EB,
                        ],
                        stg[:],
                    )

    nc.compile()
    return nc


_PROGRAM_CACHE = {}


def _get_program() -> bass.Bass:
    if "nc" not in _PROGRAM_CACHE:
        _PROGRAM_CACHE["nc"] = build_program()
    return _PROGRAM_CACHE["nc"]


def _make_mask() -> np.ndarray:
    # 0/1 triangular boundary mask for diagonal score tiles, applied post-exp:
    # within the boundary 128-chunk, valid iff local col j >= partition p.
    p = np.arange(128)[:, None]
    j = np.arange(128)[None, :]
    return np.where(j >= p, 1.0, 0.0).astype(ml_dtypes.bfloat16)


def prepare_inputs(
    hidden_states, w_qkv_a, q_a_gamma, w_q_b, kv_a_gamma, w_kv_b, w_o, b_o
):
    """Host-side prep: fold gammas + attention scale into B weights, cast to
    bf16, slice per core."""
    bf = ml_dtypes.bfloat16
    hs = np.asarray(hidden_states, np.float32).reshape(S, HID)
    scale = float(Q_HEAD) ** -0.5
    wqb_eff = (
        np.asarray(w_q_b, np.float32)
        * np.asarray(q_a_gamma, np.float32)[:, None]
        * scale
    )
    wkvb_eff = (
        np.asarray(w_kv_b, np.float32) * np.asarray(kv_a_gamma, np.float32)[:, None]
    )
    wa_bf = np.asarray(w_qkv_a, np.float32).astype(bf)
    hs_bf = hs.astype(bf)
    mask = _make_mask()
    ones = np.ones((128, 1), bf)
    onesr = np.ones((1, 128), bf)

    wqb_r = wqb_eff.reshape(Q_LORA, H, Q_HEAD)
    wkvb_r = wkvb_eff.reshape(KV_LORA, H, NOPE + V_DIM)
    wo_r = np.asarray(w_o, np.float32).reshape(H, V_DIM, HID)

    in_maps = []
    for c in range(NCORES):
        # feature-major (transposed) slice: [HID, SL]
        hsl = np.ascontiguousarray(hs_bf[c * SL : (c + 1) * SL].T)
        wqb_c = wqb_r[:, c * HL : (c + 1) * HL]  # [Q_LORA, HL, 192]
        # pack columns: A parts (128 each) for all heads, then B parts (64)
        wqb_packed = np.concatenate(
            [wqb_c[:, h, :NOPE] for h in range(HL)]
            + [wqb_c[:, h, NOPE:] for h in range(HL)],
            axis=1,
        )
        wqb_cc = np.ascontiguousarray(wqb_packed.astype(bf))
        wkvb_c = np.ascontiguousarray(
            wkvb_r[:, c * HL : (c + 1) * HL]
            .reshape(KV_LORA, HL * (NOPE + V_DIM))
            .astype(bf)
        )
        wo_c = np.ascontiguousarray(
            wo_r[c * HL : (c + 1) * HL].reshape(HL * V_DIM, HID).astype(bf)
        )
        in_maps.append(
            {
                "hid": hsl,
                "wa": wa_bf,
                "wqb": wqb_cc,
                "wkvb": wkvb_c,
                "wo": wo_c,
                "mask": mask,
                "ones": ones,
                "onesr": onesr,
            }
        )
    return in_maps


def kernel(**inputs) -> np.ndarray:
    in_maps = prepare_inputs(**inputs)
    nc = _get_program()
    res = run_bass_kernel_spmd(nc, in_maps, list(range(NCORES)))
    out = np.zeros((S, HID), np.float32)
    for r in res.results:
        out += np.asarray(r["out"], np.float32)
    out = out + np.asarray(inputs["b_o"], np.float32)[None, :]
    return out.reshape(1, S, HID)


# revision 51
# speedup vs baseline: 1.0421x; 1.0265x over previous
"""
MLA attention (DeepSeek-style) on 8 TRN2 NeuronCores.

Sharding:
  phase 1 (LoRA-A projection + RMSNorm): sharded over sequence (256 rows/core),
    result transposed to feature-major and AllGathered (bf16 latents).
    The kv+rope latent columns are computed first and gathered in an early
    collective that overlaps the rest of phase 1; the q latents follow.
  phase 2 (q/kv up-proj, attention, o_proj): sharded over heads (4 heads/core),
    w_o input-dim sharded; partial outputs summed on the host (the all-reduce).

All heavy matmuls run in bf16 with fp32 PSUM accumulation.
Everything feature-major ("X^T" layout [feature, seq]) in phase 2 so no big
transposes are needed.

v2 changes vs baseline:
  - DMA queue spreading: weight streaming on sync+vector, collective staging
    on gpsimd (so gathers fire ~100us earlier), gathered latents pulled in on
    scalar+sync.
  - softmax row-sums via DVE accumulation of exp tiles + ONE M=1 matmul per
    (head, sq-block) instead of one per score tile (-74K PE columns).
  - the 1/rowsum broadcast matmul runs in bf16 (was fp32 = 4x column rate),
    reciprocal via the fast DVE approximation.
  - causal diagonal tiles trimmed to their valid column range (scores, exp,
    AV); the boundary 128-wide chunk is masked post-exp with a 0/1 bf16 mask.
  - rope-part q up-projection packed across head pairs (M=128 instead of 64).
  - partial outputs written bf16 (halves the output stream).
"""

import os
import sys
from contextlib import ExitStack

import numpy as np

for _p in ("/opt/trn_rl_repo", "/root/.axon_site/_ro/trn_rl_repo"):
    if os.path.isdir(_p) and _p not in sys.path:
        sys.path.insert(0, _p)

import ml_dtypes  # noqa: E402

import concourse.bacc as bacc  # noqa: E402
import concourse.bass as bass  # noqa: E402
import concourse.mybir as mybir  # noqa: E402
import concourse.tile as tile  # noqa: E402
from concourse.bass_utils import run_bass_kernel_spmd  # noqa: E402
from concourse.masks import make_identity  # noqa: E402

# ---------------------------------------------------------------- constants
NCORES = 8
S = 2048
SL = S // NCORES  # 256 local rows in phase 1
HID = 4096
Q_LORA = 1536
KV_LORA = 512
ROPE = 64
C = Q_LORA + KV_LORA + ROPE  # 2112
CKV_R = KV_LORA + ROPE  # 576 kv+rope latent cols
NOPE = 128
V_DIM = 128
H = 32
HL = H // NCORES  # 4 local heads
Q_HEAD = NOPE + ROPE  # 192
EPS = 1e-6

F32 = mybir.dt.float32
BF16 = mybir.dt.bfloat16

CQ_TILES = Q_LORA // 128  # 12
CKV_TILES = KV_LORA // 128  # 4
HT_TILES = HID // 128  # 32
S_TILES = S // 128  # 16
SQB = 512
NSQB = S // SQB  # 4
EB = 512
NEB = HID // EB  # 8

# phase-1 column blocks, q first: each q block's (unnormalized) gather fires
# as soon as the block finishes, so all three q gathers + the kv gather
# pipeline down the single CC stream while the PE works through the
# a-projection and q up-projection.
# All weight streaming lives on the sync DGE queue: its trigger instructions
# carry buffer-reuse waits paced by PE progress, which must never sit in
# front of the scalar (ACT) queue's compute.
CBLOCKS = [
    (0, 512, 0),
    (512, 512, 0),
    (1024, 512, 0),
    (1536, 288, 0),
    (1824, 288, 0),
]
# feature tiles of the kv+rope latent block: 4x128 (ckv) + 1x64 (rope)
KV_CT = [(0, 128), (128, 128), (256, 128), (384, 128), (512, 64)]  # rel to 1536


# ---------------------------------------------------------------- program
def build_program() -> bass.Bass:
    nc = bacc.Bacc(
        "TRN2",
        target_bir_lowering=False,
        debug=False,
        num_devices=NCORES,
    )

    # hidden states arrive pre-transposed (feature-major) from the host, so
    # no on-device transposes are needed before the a-projection
    hid_d = nc.declare_dram_parameter("hid", [HID, SL], BF16, isOutput=False)
    wa_d = nc.declare_dram_parameter("wa", [HID, C], BF16, isOutput=False)
    wqb_d = nc.declare_dram_parameter("wqb", [Q_LORA, HL * Q_HEAD], BF16, isOutput=False)
    wkvb_d = nc.declare_dram_parameter(
        "wkvb", [KV_LORA, HL * (NOPE + V_DIM)], BF16, isOutput=False
    )
    wo_d = nc.declare_dram_parameter("wo", [HL * V_DIM, HID], BF16, isOutput=False)
    mask_d = nc.declare_dram_parameter("mask", [128, 128], BF16, isOutput=False)
    ones_d = nc.declare_dram_parameter("ones", [128, 1], BF16, isOutput=False)
    onesr_d = nc.declare_dram_parameter("onesr", [1, 128], BF16, isOutput=False)
    out_d = nc.declare_dram_parameter("out", [S, HID], BF16, isOutput=True)

    # collective bounce buffers (internal DRAM). The q latents gather in two
    # pipelined halves so the first half lands before the up-projection
    # needs it.
    NQG = 3  # q gathers: one per a-proj block, fired as each block finishes
    QHT = CQ_TILES // NQG  # 4 tiles per gather
    QH = 128 * QHT  # 512 latent rows per gather
    cc_in_kv = nc.dram_tensor("cc_in_kv", [CKV_R, SL], BF16)
    cc_out_kv = nc.dram_tensor("cc_out_kv", [NCORES, CKV_R, SL], BF16, addr_space="Shared")
    # the q latents gather UNNORMALIZED (the 1/rms row scale is applied after
    # the up-projection); the last gather carries one extra row with the
    # per-row inv-rms factors
    cc_in_q = [
        nc.dram_tensor(f"cc_in_q{i}", [QH + (1 if i == NQG - 1 else 0), SL], BF16)
        for i in range(NQG)
    ]
    cc_out_q = [
        nc.dram_tensor(
            f"cc_out_q{i}",
            [NCORES, QH + (1 if i == NQG - 1 else 0), SL],
            BF16,
            addr_space="Shared",
        )
        for i in range(NQG)
    ]

    with tile.TileContext(nc, num_cores=NCORES) as tc, ExitStack() as stack:
        # ---------------- small persistent constants
        misc = stack.enter_context(tc.tile_pool(name="misc", bufs=1))
        ident = misc.tile([128, 128], BF16, tag="ident", name="ident")
        make_identity(nc, ident[:])
        ones_sb = misc.tile([128, 1], BF16, tag="ones", name="ones")
        mask_sb = misc.tile([128, 128], BF16, tag="mask", name="mask")
        onesr_sb = misc.tile([1, 128], BF16, tag="onesr", name="onesr")
        eps_sb = misc.tile([128, 1], F32, tag="eps", name="eps")
        nc.gpsimd.memset(eps_sb[:], EPS)

        # phase-2 weights + kv latents: allocated before phase 1 so their DMAs
        # prefetch during phase-1 compute (the dma_start calls are emitted
        # inside phase 1, after the hidden-state loads, so the scalar queue
        # delivers hid first).
        wkvb_pool = stack.enter_context(tc.tile_pool(name="wkvb", bufs=1))
        wkvb_sb = [
            wkvb_pool.tile(
                [128, HL * (NOPE + V_DIM)], BF16, tag=f"wkvb{kt}", name=f"wkvb{kt}"
            )
            for kt in range(CKV_TILES)
        ]
        wqb_pool = stack.enter_context(tc.tile_pool(name="wqb", bufs=1))
        wqb_sb = [
            wqb_pool.tile([128, HL * Q_HEAD], BF16, tag=f"wqb{kt}", name=f"wqb{kt}")
            for kt in range(CQ_TILES)
        ]
        latkv = stack.enter_context(tc.tile_pool(name="latkv", bufs=1))
        latkv_sb = [
            latkv.tile([w, S], BF16, tag=f"latkv{i}", name=f"latkv{i}")
            for i, (_, w) in enumerate(KV_CT)
        ]
        kpeT = latkv_sb[-1]  # [64, S]

        # ---------------- phase 1: a-projection on local rows
        with ExitStack() as p1:
            wa_pool = p1.enter_context(tc.tile_pool(name="wa", bufs=2))
            p1_pool = p1.enter_context(tc.tile_pool(name="p1", bufs=1))
            hidT = [
                p1_pool.tile([128, SL], BF16, tag=f"hidT{ht}", name=f"hidT{ht}")
                for ht in range(HT_TILES)
            ]
            # feature-major hidden tiles all on the scalar queue (2MB total,
            # streamed in consumption order) so the sync queue's first bytes
            # are the kv-block weights the first matmuls also need
            for ht in range(HT_TILES):
                nc.scalar.dma_start(hidT[ht][:], hid_d[ht * 128 : (ht + 1) * 128, :])
            # small constants + phase-2 up-proj weights on the gpsimd queue:
            # ungated triggers, done well before the kv staging needs it
            nc.gpsimd.dma_start(mask_sb[:], mask_d[:])
            nc.gpsimd.dma_start(ones_sb[:], ones_d[:])
            nc.gpsimd.dma_start(onesr_sb[:], onesr_d[:])
            for kt in range(CKV_TILES):
                nc.gpsimd.dma_start(
                    wkvb_sb[kt][:], wkvb_d[kt * 128 : (kt + 1) * 128, :]
                )
            for kt in range(CQ_TILES):
                nc.gpsimd.dma_start(wqb_sb[kt][:], wqb_d[kt * 128 : (kt + 1) * 128, :])
            lat_sb = [
                p1_pool.tile([128, C], BF16, tag=f"lat{s2}", name=f"lat{s2}")
                for s2 in range(2)
            ]
            stat = p1_pool.tile([128, 12], F32, tag="stat", name="stat")
            # local latents^T staging (feature-major, [*, SL])
            latTq_loc = [
                p1_pool.tile([128, SL], BF16, tag=f"latTq{ct}", name=f"latTq{ct}")
                for ct in range(CQ_TILES)
            ]
            latTkv_loc = [
                p1_pool.tile([w, SL], BF16, tag=f"latTkv{i}", name=f"latTkv{i}")
                for i, (_, w) in enumerate(KV_CT)
            ]

            tps_pool = p1.enter_context(tc.tile_pool(name="tps", bufs=2, space="PSUM"))
            psum1 = p1.enter_context(tc.tile_pool(name="psum1", bufs=5, space="PSUM"))

            def load_block(c0, cw, q):
                wa_t = []
                pfx = "wakv" if c0 >= Q_LORA else "wa"
                eng = nc.sync if q == 0 else nc.scalar
                for ht in range(HT_TILES):
                    t = wa_pool.tile(
                        [128, cw], BF16, tag=f"{pfx}{ht}", name=f"wa{ht}_{c0}"
                    )
                    eng.dma_start(
                        t[:], wa_d[ht * 128 : (ht + 1) * 128, c0 : c0 + cw]
                    )
                    wa_t.append(t)
                return wa_t

            wa_tiles = {(c0, cw): load_block(c0, cw, q) for c0, cw, q in CBLOCKS}

            def fused_block(c0, cw):
                """fused[:, c0:c0+cw] = hidden @ w_qkv_a[:, c0:c0+cw] (both s-tiles)"""
                wa_t = wa_tiles[(c0, cw)]
                for s2 in range(2):
                    pf = psum1.tile([128, cw], F32, tag="pf", name=f"pf{c0}_{s2}")
                    for ht in range(HT_TILES):
                        nc.tensor.matmul(
                            pf[:],
                            hidT[ht][:, s2 * 128 : (s2 + 1) * 128],
                            wa_t[ht][:],
                            start=(ht == 0),
                            stop=(ht == HT_TILES - 1),
                        )
                    nc.scalar.copy(lat_sb[s2][:, c0 : c0 + cw], pf[:])

            def transpose_lat(src_col, w, dst):
                """dst[:, s2*128...] = lat_sb[s2][:, src_col:src_col+w]ᵀ"""
                for s2 in range(2):
                    pt = tps_pool.tile([128, 128], BF16, tag="tps", name="tpsl")
                    nc.tensor.transpose(
                        pt[:w, :], lat_sb[s2][:, src_col : src_col + w], ident[:]
                    )
                    nc.vector.tensor_copy(
                        dst[:, s2 * 128 : (s2 + 1) * 128], pt[:w, :]
                    )

            def rms_square(col0, ch, stat_base):
                """Accumulate sum-of-squares of lat_sb[:, col0+512ch : ...]."""
                for s2 in range(2):
                    sq = psum1.tile([128, 512], F32, tag="pf", name=f"sq{s2}_{ch}")
                    nc.scalar.activation(
                        sq[:],
                        lat_sb[s2][:, col0 + ch * 512 : col0 + (ch + 1) * 512],
                        mybir.ActivationFunctionType.Square,
                        accum_out=stat[:, stat_base + 3 * s2 + ch : stat_base + 3 * s2 + ch + 1],
                    )

            def rms_finalize(col0, ncols, stat_base, scale=True):
                """Compute per-row 1/rms into stat[:, stat_base+3*s2+2]; if
                `scale`, also scale lat_sb[:, col0:col0+ncols] in place."""
                nch = ncols // 512
                for s2 in range(2):
                    sb = stat_base + 3 * s2
                    for ch in range(1, nch):
                        nc.vector.tensor_add(
                            stat[:, sb : sb + 1],
                            stat[:, sb : sb + 1],
                            stat[:, sb + ch : sb + ch + 1],
                        )
                    nc.scalar.activation(
                        stat[:, sb + 1 : sb + 2],
                        stat[:, sb : sb + 1],
                        mybir.ActivationFunctionType.Sqrt,
                        scale=1.0 / ncols,
                        bias=eps_sb[:],
                    )
                    nc.vector.reciprocal(
                        stat[:, sb + 2 : sb + 3],
                        stat[:, sb + 1 : sb + 2],
                    )
                    if scale:
                        nc.scalar.activation(
                            lat_sb[s2][:, col0 : col0 + ncols],
                            lat_sb[s2][:, col0 : col0 + ncols],
                            mybir.ActivationFunctionType.Copy,
                            scale=stat[:, sb + 2 : sb + 3],
                        )

            # ---- q blocks first. Each block is transposed, staged, and
            # gathered UNNORMALIZED as soon as it finishes (the 1/rms row
            # scale is folded into the up-projection's psum->sbuf copies), so
            # the gathers pipeline down the CC stream while the PE continues
            # the a-projection.
            def stage_q_block(g, gather):
                for ctg in range(QHT):
                    ct = g * QHT + ctg
                    transpose_lat(ct * 128, 128, latTq_loc[ct])
                    nc.gpsimd.dma_start(
                        cc_in_q[g][ctg * 128 : (ctg + 1) * 128, :],
                        latTq_loc[ct][:],
                    )
                if gather:
                    nc.gpsimd.collective_compute(
                        "AllGather",
                        mybir.AluOpType.bypass,
                        replica_groups=[list(range(NCORES))],
                        ins=[cc_in_q[g][:].opt()],
                        outs=[cc_out_q[g][:].opt()],
                    )

            fused_block(0, 512)
            rms_square(0, 0, 6)
            stage_q_block(0, gather=True)
            fused_block(512, 512)
            rms_square(0, 1, 6)
            stage_q_block(1, gather=True)
            fused_block(1024, 512)
            rms_square(0, 2, 6)
            stage_q_block(2, gather=False)
            rms_finalize(0, Q_LORA, 6, scale=False)

            # ---- kv + rope blocks last; their gather overlaps the q up-proj
            fused_block(1536, 288)

            # inv-rms row rides along in the last q gather: stat cols (8, 11)
            # -> bf16 [128, 2] -> transpose -> [2, 128] -> one DRAM row.
            # Emitted between the kv blocks so the PE transpose has cover for
            # the finalize chain.
            rtmp = p1_pool.tile([128, 2], BF16, tag="rtmp", name="rtmp")
            nc.vector.tensor_copy(rtmp[:, 0:1], stat[:, 8:9])
            nc.vector.tensor_copy(rtmp[:, 1:2], stat[:, 11:12])
            rpt = tps_pool.tile([128, 128], BF16, tag="tps", name="rpt")
            nc.tensor.transpose(rpt[:2, :], rtmp[:], ident[:])
            rT = p1_pool.tile([2, 128], BF16, tag="rT", name="rT")
            nc.vector.tensor_copy(rT[:], rpt[:2, :])
            nc.gpsimd.dma_start(
                cc_in_q[2][QH : QH + 1, :].rearrange(
                    "one (p f) -> (one p) f", p=2
                ),
                rT[:],
            )
            nc.gpsimd.collective_compute(
                "AllGather",
                mybir.AluOpType.bypass,
                replica_groups=[list(range(NCORES))],
                ins=[cc_in_q[2][:].opt()],
                outs=[cc_out_q[2][:].opt()],
            )

            fused_block(1824, 288)
            rms_square(Q_LORA, 0, 0)
            rms_finalize(Q_LORA, KV_LORA, 0)
            # staging on the otherwise-idle gpsimd queue so it never waits
            # behind weight streaming on the HW DGE queues
            for i, (rel, w) in enumerate(KV_CT):
                transpose_lat(Q_LORA + rel, w, latTkv_loc[i])
                nc.gpsimd.dma_start(
                    cc_in_kv[rel : rel + w, :], latTkv_loc[i][:]
                )
            nc.gpsimd.collective_compute(
                "AllGather",
                mybir.AluOpType.bypass,
                replica_groups=[list(range(NCORES))],
                ins=[cc_in_kv[:].opt()],
                outs=[cc_out_kv[:].opt()],
            )

        # ---------------- phase 2
        kvpool = stack.enter_context(tc.tile_pool(name="kvpool", bufs=1))
        knopeT = [
            kvpool.tile([128, S], BF16, tag=f"knopeT{h}", name=f"knopeT{h}")
            for h in range(HL)
        ]
        v_sb = [
            kvpool.tile([128, HL * V_DIM], BF16, tag=f"v{st}", name=f"v{st}")
            for st in range(S_TILES)
        ]

        qT = stack.enter_context(tc.tile_pool(name="qT", bufs=1))
        qTA = [qT.tile([128, S], BF16, tag=f"qTA{h}", name=f"qTA{h}") for h in range(HL)]
        qTB = [qT.tile([64, S], BF16, tag=f"qTB{h}", name=f"qTB{h}") for h in range(HL)]
        outT_pool = stack.enter_context(tc.tile_pool(name="outT", bufs=1))
        outT = [
            outT_pool.tile([128, S], BF16, tag=f"outT{h}", name=f"outT{h}")
            for h in range(HL)
        ]

        # q^T (scoped: big q-latents released after). wqb column layout (host
        # side): [A(h0)|A(h1)|A(h2)|A(h3)|B(h0)|B(h1)|B(h2)|B(h3)] so the
        # 64-wide rope (B) parts of head pairs pack into M=128 matmuls.
        with ExitStack() as p2q:
            latq = p2q.enter_context(tc.tile_pool(name="latq", bufs=1))
            latq_sb = [
                latq.tile([128, S], BF16, tag=f"latq{ct}", name=f"latq{ct}")
                for ct in range(CQ_TILES)
            ]
            # all latq landings on the sync queue (idle once the weights are
            # through); each lands as its gather completes
            for g in range(NQG):
                cc_q_view = cc_out_q[g][:].rearrange("j c s -> c j s")
                for ctg in range(QHT):
                    ct = g * QHT + ctg
                    nc.sync.dma_start(
                        latq_sb[ct][:].rearrange("c (j s) -> c j s", j=NCORES),
                        cc_q_view[ctg * 128 : (ctg + 1) * 128],
                    )
            # gathered kv latents land behind them (needed later, for the
            # k/v up-projections that now follow the q up-projection)
            cc_kv_view = cc_out_kv[:].rearrange("j c s -> c j s")
            for i, (rel, w) in enumerate(KV_CT):
                nc.sync.dma_start(
                    latkv_sb[i][:].rearrange("c (j s) -> c j s", j=NCORES),
                    cc_kv_view[rel : rel + w],
                )
            # inv-rms row -> broadcast tile [128, S] used to scale the
            # up-projected q as it leaves PSUM
            bcr_pool = p2q.enter_context(tc.tile_pool(name="bcr", bufs=1))
            r_sb = bcr_pool.tile([1, S], BF16, tag="rsb", name="rsb")
            nc.scalar.dma_start(
                r_sb[:].rearrange("one (j s) -> one j s", j=NCORES),
                cc_out_q[2][:, QH : QH + 1, :].rearrange("j one s -> one j s"),
            )
            bc_r = bcr_pool.tile([128, S], F32, tag="bcr", name="bcr")
            pq_pool = p2q.enter_context(tc.tile_pool(name="pq", bufs=8, space="PSUM"))

            def build_bc_r():
                for blk in range(4):
                    bcp = pq_pool.tile([128, 512], F32, tag="pq", name=f"bcp{blk}")
                    nc.tensor.matmul(
                        bcp[:],
                        onesr_sb[:],
                        r_sb[:, blk * 512 : (blk + 1) * 512],
                        start=True,
                        stop=True,
                    )
                    nc.scalar.copy(bc_r[:, blk * 512 : (blk + 1) * 512], bcp[:])

            def qup_pass(col0, copies, post_mm=None):
                pqs = [
                    pq_pool.tile([128, SQB], F32, tag="pq", name=f"pq{col0}_{sqb}")
                    for sqb in range(NSQB)
                ]
                for kt in range(CQ_TILES):
                    for sqb in range(NSQB):
                        nc.tensor.matmul(
                            pqs[sqb][:],
                            wqb_sb[kt][:, col0 : col0 + 128],
                            latq_sb[kt][:, sqb * SQB : (sqb + 1) * SQB],
                            start=(kt == 0),
                            stop=(kt == CQ_TILES - 1),
                        )
                if post_mm is not None:
                    post_mm()
                for sqb in range(NSQB):
                    for dst, (r0, r1) in copies:
                        # psum -> sbuf copy fused with the 1/rms row scale
                        nc.vector.tensor_mul(
                            dst[:, sqb * SQB : (sqb + 1) * SQB],
                            pqs[sqb][r0:r1, :],
                            bc_r[r0:r1, sqb * SQB : (sqb + 1) * SQB],
                        )

            # head 0's A and B parts first so attention's first tiles aren't
            # gated on the last psum copies; the bc_r build slots in after the
            # first pass's matmuls, by which time the inv-rms row has landed
            qup_pass(0 * 128, [(qTA[0], (0, 128))], post_mm=build_bc_r)
            qup_pass(HL * 128, [(qTB[0], (0, 64)), (qTB[1], (64, 128))])
            qup_pass(1 * 128, [(qTA[1], (0, 128))])
            qup_pass(HL * 128 + 128, [(qTB[2], (0, 64)), (qTB[3], (64, 128))])
            qup_pass(2 * 128, [(qTA[2], (0, 128))])
            qup_pass(3 * 128, [(qTA[3], (0, 128))])

        with ExitStack() as p2kv:
            # ---- k_nope^T and V up-projections (after the q up-proj: their
            # gather is the last one down the CC stream)
            pkv_pool = p2kv.enter_context(tc.tile_pool(name="pkv", bufs=4, space="PSUM"))
            for h in range(HL):
                for skb in range(NSQB):
                    pk = pkv_pool.tile([128, SQB], F32, tag="pkv", name="pk")
                    for kt in range(CKV_TILES):
                        nc.tensor.matmul(
                            pk[:],
                            wkvb_sb[kt][
                                :, h * (NOPE + V_DIM) : h * (NOPE + V_DIM) + NOPE
                            ],
                            latkv_sb[kt][:, skb * SQB : (skb + 1) * SQB],
                            start=(kt == 0),
                            stop=(kt == CKV_TILES - 1),
                        )
                    nc.scalar.copy(knopeT[h][:, skb * SQB : (skb + 1) * SQB], pk[:])
            for st in range(S_TILES):
                pv = pkv_pool.tile([128, HL * V_DIM], F32, tag="pkv", name="pv")
                for kt in range(CKV_TILES):
                    rhs = wkvb_sb[kt][:].rearrange("c (h d) -> c h d", h=HL)[:, :, NOPE:]
                    nc.tensor.matmul(
                        pv[:],
                        latkv_sb[kt][:, st * 128 : (st + 1) * 128],
                        rhs,
                        start=(kt == 0),
                        stop=(kt == CKV_TILES - 1),
                    )
                nc.scalar.copy(v_sb[st][:], pv[:])

        wo_pool = stack.enter_context(tc.tile_pool(name="wo", bufs=1))
        wo_sb = [
            wo_pool.tile([128, HID], BF16, tag=f"wo{h}", name=f"wo{h}")
            for h in range(HL)
        ]

        # ---------------- attention (causal, block-skipped, diagonal-trimmed)
        # Score tiles: for (h, bq), k-tiles tk = 0..4(bq+1)-1; the 4 diagonal
        # tiles (d = tk-4bq >= 0) only compute their valid column range
        # [128d, 512). exp'd tiles are masked post-exp (x 0/1 bf16 mask) on
        # their boundary 128-chunk, accumulated into `acc` on the DVE; the
        # softmax denominator is ONE [128,1]-ones matmul on acc per (h, bq).
        # Software-pipelined: AV matmuls trail the score tiles by two tiles;
        # the renormalization epilogue trails by one (h, bq) pair.
        with ExitStack() as p2a:
            ps_pool = p2a.enter_context(tc.tile_pool(name="ps", bufs=4, space="PSUM"))
            psum_sum_pool = p2a.enter_context(
                tc.tile_pool(name="psums", bufs=2, space="PSUM")
            )
            psum_o_pool = p2a.enter_context(
                tc.tile_pool(name="psumo", bufs=2, space="PSUM")
            )
            a_pool = p2a.enter_context(tc.tile_pool(name="apool", bufs=6))
            acc_pool = p2a.enter_context(tc.tile_pool(name="accp", bufs=2))
            bc_pool = p2a.enter_context(tc.tile_pool(name="bcpool", bufs=3))

            tile_q = []  # score tiles awaiting their AV matmuls
            ep_q = []  # pairs awaiting the renormalization epilogue

            def drain_tile():
                a, w, h, bq, tk, nk, acc, po = tile_q.pop(0)
                off = SQB - w
                nc.tensor.matmul(
                    po[:, off:SQB],
                    v_sb[tk][:, h * V_DIM : (h + 1) * V_DIM],
                    a[:, :w],
                    start=(tk == 0),
                    stop=(tk == nk - 1),
                    skip_group_check=True,
                )
                if tk == nk - 1:
                    # softmax denominator: one M=1 matmul over the DVE-summed
                    # exp accumulator, then fast reciprocal -> bf16
                    psum = psum_sum_pool.tile([1, SQB], F32, tag="psums", name="psum")
                    nc.tensor.matmul(
                        psum[:], ones_sb[:], acc[:], start=True, stop=True
                    )
                    rs32 = bc_pool.tile([1, SQB], F32, tag="rs32", name="rs32")
                    rs = bc_pool.tile([1, SQB], BF16, tag="rs", name="rs")
                    nc.vector.reciprocal_approx_fast(rs32[:], psum[:])
                    nc.vector.tensor_copy(rs[:], rs32[:])
                    ep_q.append((h, bq, po, rs))

            def drain_epilogue():
                h, bq, po, rs = ep_q.pop(0)
                bc_ps = ps_pool.tile([128, SQB], F32, tag="ps", name="bc_ps")
                nc.tensor.matmul(bc_ps[:], onesr_sb[:], rs[:], start=True, stop=True)
                bc_sb = bc_pool.tile([128, SQB], F32, tag="bc", name="bc_sb")
                nc.scalar.copy(bc_sb[:], bc_ps[:])
                nc.vector.tensor_mul(
                    outT[h][:, bq * SQB : (bq + 1) * SQB], po[:], bc_sb[:]
                )

            for h in range(HL):
                for bq in range(NSQB):
                    nk = 4 * (bq + 1)
                    acc = acc_pool.tile([128, SQB], BF16, tag="acc", name="acc")
                    po = psum_o_pool.tile([128, SQB], F32, tag="psumo", name="po")
                    for tk in range(nk):
                        d = tk - 4 * bq
                        w = SQB if d < 0 else SQB - 128 * d  # valid cols
                        off = SQB - w  # offset inside the sq block
                        ps = ps_pool.tile([128, SQB], F32, tag="ps", name="ps")
                        nc.tensor.matmul(
                            ps[:, :w],
                            knopeT[h][:, tk * 128 : (tk + 1) * 128],
                            qTA[h][:, bq * SQB + off : (bq + 1) * SQB],
                            start=True,
                            stop=False,
                        )
                        nc.tensor.matmul(
                            ps[:, :w],
                            kpeT[:, tk * 128 : (tk + 1) * 128],
                            qTB[h][:, bq * SQB + off : (bq + 1) * SQB],
                            start=False,
                            stop=True,
                        )
                        a = a_pool.tile([128, SQB], BF16, tag="a", name="a")
                        nc.scalar.activation(
                            a[:, :w], ps[:, :w], mybir.ActivationFunctionType.Exp
                        )
                        if d >= 0:
                            # boundary chunk: zero the causally-invalid part
                            nc.vector.tensor_mul(
                                a[:, :128], a[:, :128], mask_sb[:]
                            )
                        if tk == 0:
                            nc.vector.tensor_copy(acc[:], a[:])
                        else:
                            nc.vector.tensor_add(
                                acc[:, off:SQB], acc[:, off:SQB], a[:, :w]
                            )
                        tile_q.append((a, w, h, bq, tk, nk, acc, po))
                        while len(tile_q) > 2:
                            drain_tile()
                        while len(ep_q) > 1:
                            drain_epilogue()
                if h == 0:
                    # o_proj weights stream during attention, after the q
                    # gathers are off the wire
                    for hh in range(HL):
                        nc.scalar.dma_start(
                            wo_sb[hh][:], wo_d[hh * 128 : (hh + 1) * 128, :]
                        )
            while tile_q:
                drain_tile()
            while ep_q:
                drain_epilogue()

        # ---------------- o_proj (partial: summed across cores on host)
        with ExitStack() as p2o:
            pe_pool = p2o.enter_context(tc.tile_pool(name="pe", bufs=4, space="PSUM"))
            stage_pool = p2o.enter_context(tc.tile_pool(name="stage", bufs=4))
            for st in range(S_TILES):
                for half in range(2):
                    stg = stage_pool.tile([128, 4 * EB], BF16, tag="stage", name="stg")
                    for ebl in range(4):
                        eb = half * 4 + ebl
                        pe = pe_pool.tile([128, EB], F32, tag="pe", name="pe")
                        for h in range(HL):
                            nc.tensor.matmul(
                                pe[:],
                                outT[h][:, st * 128 : (st + 1) * 128],
                                wo_sb[h][:, eb * EB : (eb + 1) * EB],
                                start=(h == 0),
                                stop=(h == HL - 1),
                            )
                        if ebl % 2 == 0:
                            nc.vector.tensor_copy(
                                stg[:, ebl * EB : (ebl + 1) * EB], pe[:]
                            )
                        else:
                            nc.scalar.copy(
                                stg[:, ebl * EB : (ebl + 1) * EB], pe[:]
                            )
                    nc.sync.dma_start(
                        out_d[
                            st * 128 : (st + 1) * 128,
                            half * 4 * EB : (half + 1) * 4 * EB,
                        ],
                        stg[:],
                    )

    nc.compile()
    return nc


_PROGRAM_CACHE = {}


def _get_program() -> bass.Bass:
    if "nc" not in _PROGRAM_CACHE:
        _PROGRAM_CACHE["nc"] = build_program()
    return _PROGRAM_CACHE["nc"]


def _make_mask() -> np.ndarray:
    # 0/1 triangular boundary mask for diagonal score tiles, applied post-exp:
    # within the boundary 128-chunk, valid iff local col j >= partition p.
    p = np.arange(128)[:, None]
    j = np.arange(128)[None, :]
    return np.where(j >= p, 1.0, 0.0).astype(ml_dtypes.bfloat16)


def prepare_inputs(
    hidden_states, w_qkv_a, q_a_gamma, w_q_b, kv_a_gamma, w_kv_b, w_o, b_o
):
    """Host-side prep: fold gammas + attention scale into B weights, cast to
    bf16, slice per core."""
    bf = ml_dtypes.bfloat16
    hs = np.asarray(hidden_states, np.float32).reshape(S, HID)
    scale = float(Q_HEAD) ** -0.5
    wqb_eff = (
        np.asarray(w_q_b, np.float32)
        * np.asarray(q_a_gamma, np.float32)[:, None]
        * scale
    )
    wkvb_eff = (
        np.asarray(w_kv_b, np.float32) * np.asarray(kv_a_gamma, np.float32)[:, None]
    )
    wa_bf = np.asarray(w_qkv_a, np.float32).astype(bf)
    hs_bf = hs.astype(bf)
    mask = _make_mask()
    ones = np.ones((128, 1), bf)
    onesr = np.ones((1, 128), bf)

    wqb_r = wqb_eff.reshape(Q_LORA, H, Q_HEAD)
    wkvb_r = wkvb_eff.reshape(KV_LORA, H, NOPE + V_DIM)
    wo_r = np.asarray(w_o, np.float32).reshape(H, V_DIM, HID)

    in_maps = []
    for c in range(NCORES):
        # feature-major (transposed) slice: [HID, SL]
        hsl = np.ascontiguousarray(hs_bf[c * SL : (c + 1) * SL].T)
        wqb_c = wqb_r[:, c * HL : (c + 1) * HL]  # [Q_LORA, HL, 192]
        # pack columns: A parts (128 each) for all heads, then B parts (64)
        wqb_packed = np.concatenate(
            [wqb_c[:, h, :NOPE] for h in range(HL)]
            + [wqb_c[:, h, NOPE:] for h in range(HL)],
            axis=1,
        )
        wqb_cc = np.ascontiguousarray(wqb_packed.astype(bf))
        wkvb_c = np.ascontiguousarray(
            wkvb_r[:, c * HL : (c + 1) * HL]
            .reshape(KV_LORA, HL * (NOPE + V_DIM))
            .astype(bf)
        )
        wo_c = np.ascontiguousarray(
            wo_r[c * HL : (c + 1) * HL].reshape(HL * V_DIM, HID).astype(bf)
        )
        in_maps.append(
            {
                "hid": hsl,
                "wa": wa_bf,
                "wqb": wqb_cc,
                "wkvb": wkvb_c,
                "wo": wo_c,
                "mask": mask,
                "ones": ones,
                "onesr": onesr,
            }
        )
    return in_maps


def kernel(**inputs) -> np.ndarray:
    in_maps = prepare_inputs(**inputs)
    nc = _get_program()
    res = run_bass_kernel_spmd(nc, in_maps, list(range(NCORES)))
    out = np.zeros((S, HID), np.float32)
    for r in res.results:
        out += np.asarray(r["out"], np.float32)
    out = out + np.asarray(inputs["b_o"], np.float32)[None, :]
    return out.reshape(1, S, HID)
